# revision 25
# baseline (speedup 1.0000x reference)
"""KimiDeltaAttention — Trainium2 Bass kernel, 8-core head-sharded SPMD.

Each core handles 2 of 16 heads (256 of 2048 channels):
  - q/k/v projections + causal depthwise conv + silu  (bf16 matmuls, fp32 psum)
  - decay gate g = -exp(A_log) * softplus(low-rank proj + dt_bias)   (fp32)
  - l2norm(q)*D^-0.5, l2norm(k)
  - chunked gated-delta-rule recurrence (chunk C=128, sub-chunk SC=4,
    block-start boundary factorization so every exp arg is <= +66; W/P via
    32 row-block matmuls; (I + W diag(beta))^{-1} via Neumann doubling)
  - gated RMSNorm (sigmoid low-rank gate), output projection partial
Host: shards inputs, computes beta (tiny), sums the 8 partial outputs.

Shapes hardcoded: B=2, T=2048, HID=2048, H=16, D=128, K=4.
"""

import os

os.environ.setdefault("JAX_COMPILATION_CACHE_DIR", "/root/jax_cache")
os.environ.setdefault("JAX_PERSISTENT_CACHE_MIN_ENTRY_SIZE_BYTES", "-1")
os.environ.setdefault("JAX_PERSISTENT_CACHE_MIN_COMPILE_TIME_SECS", "0")

import sys

if "/opt/trn_rl_repo" not in sys.path:
    sys.path.insert(0, "/opt/trn_rl_repo")

import numpy as np
import ml_dtypes

bf16 = ml_dtypes.bfloat16

B, T, HID = 2, 2048, 2048
H, D = 16, 128
P = H * D
N = B * T            # 4096 tokens
C = 128              # chunk length
SC = 4               # row-block
WIN = 64             # col window per row-block
NBLK = C // SC       # 32
NCH = T // C         # 16 chunks per sequence
EPS = 1e-6
KLEV = 4             # Neumann doubling levels (A^1..A^16)
BIGNEG_PAD = 3.0e38  # left-pad value so exp(a - pad) == 0

_CACHED = {}


def _build_nc():
    import concourse.bass as bass
    import concourse.tile as tile
    from concourse import bacc, mybir

    f32 = mybir.dt.float32
    b16 = mybir.dt.bfloat16

    nc = bacc.Bacc("TRN2", target_bir_lowering=False, debug=False, num_devices=8)

    # ---- I/O ----
    hst = nc.declare_dram_parameter("hst", [HID, N], b16, isOutput=False)          # hs^T tiled rows
    wq = nc.declare_dram_parameter("wq", [128, 16 * 256], b16, isOutput=False)     # kt-tiled W^T
    wk = nc.declare_dram_parameter("wk", [128, 16 * 256], b16, isOutput=False)
    wv = nc.declare_dram_parameter("wv", [128, 16 * 256], b16, isOutput=False)
    wga = nc.declare_dram_parameter("wga", [128, 16 * 128], b16, isOutput=False)
    wgb = nc.declare_dram_parameter("wgb", [128, 256], b16, isOutput=False)
    gs = nc.declare_dram_parameter("gs", [256, N], f32, isOutput=False)
    wo = nc.declare_dram_parameter("wo", [256, HID], b16, isOutput=False)          # rows=channels
    cwq = nc.declare_dram_parameter("cwq", [128, 8], f32, isOutput=False)
    cwk = nc.declare_dram_parameter("cwk", [128, 8], f32, isOutput=False)
    cwv = nc.declare_dram_parameter("cwv", [128, 8], f32, isOutput=False)
    betah = nc.declare_dram_parameter("betah", [2, N], f32, isOutput=False)
    lx = nc.declare_dram_parameter("lx", [128, 128], f32, isOutput=False)          # L[j,t]=1 if j<=t
    maskS = nc.declare_dram_parameter("maskS", [128, 128], b16, isOutput=False)    # strict lower
    maskI = nc.declare_dram_parameter("maskI", [128, 128], b16, isOutput=False)    # incl lower
    idb = nc.declare_dram_parameter("idb", [128, 128], b16, isOutput=False)        # identity bf16
    idf = nc.declare_dram_parameter("idf", [128, 128], f32, isOutput=False)        # identity f32
    pm1 = nc.declare_dram_parameter("pm1", [1, 2], f32, isOutput=False)            # [+1, -1]
    out = nc.declare_dram_parameter("out", [N, HID], b16, isOutput=True)

    # DRAM scratch
    qs = nc.dram_tensor("qs", [256, N], b16)
    ks = nc.dram_tensor("ks", [256, N], b16)
    vs = nc.dram_tensor("vs", [256, N], b16)
    ot = nc.dram_tensor("ot", [N, 256], b16)
    wdn = nc.dram_tensor("wdn", [4, 128, 192], b16)
    pdn = nc.dram_tensor("pdn", [4, 128, 192], b16)
    sg = nc.dram_tensor("sg", [256, N], b16)
    og = nc.dram_tensor("og", [256, N], b16)

    import bass_rust

    def pat(ap, offset_elems, dims, dtype_bytes):
        """Custom free-dim pattern on a 2D sbuf tile ap (keeps partition dim)."""
        c = ap.copy()
        part = list(c.ap)[0]
        c.ap = bass_rust.VecI64Pair([list(part)] + [list(d) for d in dims])
        c.offset = ap.offset + offset_elems
        return c

    with tile.TileContext(nc) as tc:
        with tc.tile_pool(name="consts", bufs=1) as cpool:
            lx_sb = cpool.tile([128, 128], f32)
            nc.sync.dma_start(lx_sb[:], lx[:])
            mS_sb = cpool.tile([128, 128], b16)
            nc.sync.dma_start(mS_sb[:], maskS[:])
            mI_sb = cpool.tile([128, 128], b16)
            nc.sync.dma_start(mI_sb[:], maskI[:])
            idb_sb = cpool.tile([128, 128], b16)
            nc.sync.dma_start(idb_sb[:], idb[:])
            idf_sb = cpool.tile([128, 128], f32)
            nc.sync.dma_start(idf_sb[:], idf[:])
            pm1_sb = cpool.tile([1, 2], f32)
            nc.sync.dma_start(pm1_sb[:], pm1[:])
            beta0 = cpool.tile([1, N], f32)
            nc.sync.dma_start(beta0[:], betah[0:1, :])
            beta1 = cpool.tile([1, N], f32)
            nc.sync.dma_start(beta1[:], betah[1:2, :])
            beta_t = (beta0, beta1)
            zro = cpool.tile([128, 192], b16)
            nc.gpsimd.memset(zro[:], 0.0)
            epst = cpool.tile([128, 2], f32)
            nc.gpsimd.memset(epst[:, 0:1], EPS)
            nc.gpsimd.memset(epst[:, 1:2], EPS * D)
            cw_sb = {}
            for nm, t in (("q", cwq), ("k", cwk), ("v", cwv)):
                cwt = cpool.tile([128, 8], f32, tag=f"cw{nm}")
                cw_sb[nm] = cwt
                nc.sync.dma_start(cw_sb[nm][:], t[:])

            # ================= Phase B: projections =================
            SEG = 2048 + 3
            with (
                tc.tile_pool(name="hts", bufs=1) as hpool,
                tc.tile_pool(name="wts", bufs=1) as wpool,
                tc.tile_pool(name="xp", bufs=1) as xpool,
                tc.tile_pool(name="proj", bufs=1) as ppool,
                tc.tile_pool(name="nrm", bufs=1) as npool,
                tc.tile_pool(name="projps", bufs=2, space="PSUM") as pps,
            ):
                w_sb = {}
                for nm, t in (("q", wq), ("k", wk), ("v", wv)):
                    wt_ = wpool.tile([128, 16 * 256], b16, tag=f"w{nm}")
                    w_sb[nm] = wt_
                    nc.sync.dma_start(w_sb[nm][:], t[:])
                for nm, t in (("ga", wga),):
                    wt_ = wpool.tile([128, 16 * 128], b16, tag=f"w{nm}")
                    w_sb[nm] = wt_
                    nc.sync.dma_start(w_sb[nm][:], t[:])
                for nm, t in (("gb", wgb),):
                    wt_ = wpool.tile([128, 256], b16, tag=f"w{nm}")
                    w_sb[nm] = wt_
                    nc.sync.dma_start(w_sb[nm][:], t[:])
                ones_col = wpool.tile([128, 1], b16, tag="ones")
                nc.gpsimd.memset(ones_col[:], 1.0)
                ones_row = wpool.tile([1, 128], b16, tag="onesr")
                nc.gpsimd.memset(ones_row[:], 1.0)

                xpads = {}
                for nm in ("q", "k", "v"):
                    for ch in range(2):
                        xp = xpool.tile([128, 2 * SEG], b16, tag=f"xp{nm}{ch}")
                        nc.gpsimd.memset(xp[:, 0:3], 0.0)
                        nc.gpsimd.memset(xp[:, SEG:SEG + 3], 0.0)
                        xpads[(nm, ch)] = xp
                lowr = {}
                for nm in ("ga",):
                    lrt = xpool.tile([128, N], b16, tag=f"lr{nm}")
                    lowr[nm] = lrt

                for nt in range(8):
                    hts = []
                    for kt in range(16):
                        ht = hpool.tile([128, 512], b16, tag=f"ht{kt}")
                        nc.sync.dma_start(
                            ht[:], hst[kt * 128:(kt + 1) * 128, nt * 512:(nt + 1) * 512]
                        )
                        hts.append(ht)
                    for nm in ("q", "k", "v"):
                        for ch in range(2):
                            ps = pps.tile([128, 512], f32, tag="pp")
                            for kt in range(16):
                                nc.tensor.matmul(
                                    ps[:],
                                    w_sb[nm][:, kt * 256 + ch * 128: kt * 256 + (ch + 1) * 128],
                                    hts[kt][:],
                                    start=(kt == 0), stop=(kt == 15),
                                )
                            b = nt // 4
                            col = b * SEG + 3 + (nt % 4) * 512
                            nc.scalar.copy(xpads[(nm, ch)][:, col:col + 512], ps[:])
                    for nm in ("ga",):
                        ps = pps.tile([128, 512], f32, tag="pp")
                        for kt in range(16):
                            nc.tensor.matmul(
                                ps[:], w_sb[nm][:, kt * 128:(kt + 1) * 128],
                                hts[kt][:], start=(kt == 0), stop=(kt == 15),
                            )
                        nc.scalar.copy(lowr[nm][:, nt * 512:(nt + 1) * 512], ps[:])

                # ---- gate: sg (bf16) ----
                for ch in range(2):
                    for nt in range(8):
                        ps2 = pps.tile([128, 512], f32, tag="pp")
                        nc.tensor.matmul(
                            ps2[:], w_sb["gb"][:, ch * 128:(ch + 1) * 128],
                            lowr["ga"][:, nt * 512:(nt + 1) * 512],
                            start=True, stop=True,
                        )
                        sgb = ppool.tile([128, 512], b16, tag="sgb")
                        nc.scalar.activation(
                            sgb[:], ps2[:], mybir.ActivationFunctionType.Sigmoid
                        )
                        nc.sync.dma_start(
                            sg[ch * 128:(ch + 1) * 128, nt * 512:(nt + 1) * 512], sgb[:]
                        )

                # ---- conv + silu (+ l2norm for q,k) ----
                mult = mybir.AluOpType.mult
                add = mybir.AluOpType.add
                for ch in range(2):
                    for nm, dst in (("q", qs), ("k", ks), ("v", vs)):
                        xpad = xpads[(nm, ch)]
                        wcol = cw_sb[nm][:, ch * 4:(ch + 1) * 4]
                        y = ppool.tile([128, N], b16, tag="ysb")
                        for b in range(2):
                            ysl = y[:, b * 2048:(b + 1) * 2048]
                            xb = xpad[:, b * SEG: (b + 1) * SEG]
                            nc.vector.tensor_scalar_mul(ysl, xb[:, 3:2051], wcol[:, 3:4])
                            for tau in (2, 1, 0):
                                nc.vector.scalar_tensor_tensor(
                                    ysl, xb[:, tau:tau + 2048], wcol[:, tau:tau + 1],
                                    ysl, mult, add,
                                )
                        ysil = ppool.tile([128, N], b16, tag="ysil")
                        nc.scalar.activation(
                            ysil[:], y[:], mybir.ActivationFunctionType.Silu
                        )
                        if nm == "v":
                            nc.sync.dma_start(dst[ch * 128:(ch + 1) * 128, :], ysil[:])
                            continue
                        # l2norm over d (partitions) via ones-matmul
                        ysq = ppool.tile([128, N], b16, tag="ysb")
                        nc.scalar.activation(
                            ysq[:], ysil[:], mybir.ActivationFunctionType.Square
                        )
                        ssq = npool.tile([1, N], f32, tag="ssq")
                        for nt in range(8):
                            ssp = pps.tile([1, 512], f32, tag="ssp")
                            nc.tensor.matmul(
                                ssp[:], ones_col[:],
                                ysq[:, nt * 512:(nt + 1) * 512],
                                start=True, stop=True,
                            )
                            nc.scalar.copy(ssq[:, nt * 512:(nt + 1) * 512], ssp[:])
                        scl = float(D) if nm == "q" else 1.0
                        bcol = epst[0:1, 1:2] if nm == "q" else epst[0:1, 0:1]
                        nc.scalar.activation(
                            ssq[:], ssq[:], mybir.ActivationFunctionType.Sqrt,
                            bias=bcol, scale=scl,
                        )
                        rr = npool.tile([1, N], b16, tag="rr")
                        with nc.allow_low_precision(reason="bf16 norm scale"):
                            nc.vector.reciprocal(rr[:], ssq[:])
                        yn = ppool.tile([128, N], b16, tag="ysb")
                        for nt in range(8):
                            sl = slice(nt * 512, (nt + 1) * 512)
                            bb = pps.tile([128, 512], f32, tag="pp")
                            nc.tensor.matmul(
                                bb[:], ones_row[:], rr[:, sl], start=True, stop=True
                            )
                            nc.vector.tensor_mul(yn[:, sl], ysil[:, sl], bb[:])
                        nc.sync.dma_start(dst[ch * 128:(ch + 1) * 128, :], yn[:])

            # ================= Phase C: recurrence =================
            mult = mybir.AluOpType.mult
            add = mybir.AluOpType.add
            sub = mybir.AluOpType.subtract
            AF = mybir.ActivationFunctionType
            with (
                tc.tile_pool(name="state", bufs=1) as spool,
                tc.tile_pool(name="rc", bufs=3) as rc,
                tc.tile_pool(name="rcpsf", bufs=3, space="PSUM") as rcps_f,
                tc.tile_pool(name="rcpsb", bufs=2, space="PSUM") as rcps_b,
                tc.tile_pool(name="wpps", bufs=2, space="PSUM") as wpps,
            ):
                Sf = []
                Sb = []
                for s in range(4):
                    sf = spool.tile([128, 128], f32, tag=f"Sf{s}")
                    nc.gpsimd.memset(sf[:], 0.0)
                    Sf.append(sf)
                    sbt = spool.tile([128, 128], b16, tag=f"Sb{s}")
                    nc.gpsimd.memset(sbt[:], 0.0)
                    Sb.append(sbt)

                for n in range(NCH):
                    for s in range(4):
                        b, h = s // 2, s % 2
                        t0 = b * T + n * C
                        rsl = slice(h * 128, (h + 1) * 128)
                        csl = slice(t0, t0 + C)

                        q_cm = rc.tile([128, C], b16, tag="q_cm")
                        nc.sync.dma_start(q_cm[:], qs[rsl, csl])
                        k_cm = rc.tile([128, C], b16, tag="k_cm")
                        nc.sync.dma_start(k_cm[:], ks[rsl, csl])
                        v_tm = rc.tile([128, C], b16, tag="v_tm")
                        nc.sync.dma_start_transpose(v_tm[:], vs[rsl, csl])
                        g_cm = rc.tile([128, C], f32, tag="g_cm")
                        nc.sync.dma_start(g_cm[:], gs[rsl, csl])
                        gtp = rcps_f.tile([128, C], f32, tag="psf")
                        nc.tensor.transpose(gtp[:], g_cm[:], idf_sb[:])
                        g_tm = rc.tile([128, C], f32, tag="g_tm")
                        nc.vector.tensor_copy(g_tm[:], gtp[:])

                        # beta columns: [128, 2] = (beta, -beta)
                        bps = rcps_f.tile([128, 2], f32, tag="psf")
                        nc.tensor.matmul(
                            bps[:], beta_t[h][:, csl], pm1_sb[:],
                            start=True, stop=True,
                        )
                        bcols = rc.tile([128, 2], f32, tag="bcols")
                        nc.scalar.copy(bcols[:], bps[:])

                        # cumsum c = L @ g_tm  -> c_tm -> transpose -> cpad
                        cps = rcps_f.tile([128, C], f32, tag="psf")
                        nc.tensor.matmul(cps[:], lx_sb[:], g_tm[:], start=True, stop=True)
                        c_tm = rc.tile([128, C], f32, tag="c_tm")
                        nc.scalar.copy(c_tm[:], cps[:])
                        ctp = rcps_f.tile([128, C], f32, tag="psf")
                        nc.tensor.transpose(ctp[:], c_tm[:], idf_sb[:])
                        cpad = rc.tile([128, 61 + C], f32, tag="cpad")
                        nc.gpsimd.memset(cpad[:, 0:60], BIGNEG_PAD)
                        nc.gpsimd.memset(cpad[:, 60:61], 0.0)
                        nc.vector.tensor_copy(cpad[:, 61:61 + C], ctp[:])
                        c_cm = cpad[:, 61:61 + C]

                        kqpad = rc.tile([128, 60 + C], b16, tag="kqpad")
                        nc.gpsimd.memset(kqpad[:, 0:60], 0.0)
                        nc.vector.tensor_copy(kqpad[:, 60:60 + C], k_cm[:])

                        # colD[d,(I,jj)] = c_j - a_I  (j = 4I-60+jj), 32 TS ops
                        colD = rc.tile([128, NBLK * WIN], f32, tag="colD")
                        for I in range(NBLK):
                            nc.vector.tensor_scalar_sub(
                                colD[:, I * WIN:(I + 1) * WIN],
                                cpad[:, 4 * I + 1:4 * I + 65],
                                cpad[:, 60 + 4 * I:61 + 4 * I],
                            )
                        colE = rc.tile([128, NBLK * WIN], b16, tag="colE")
                        nc.scalar.activation(colE[:], colD[:], AF.Exp, scale=-1.0)
                        k_w = pat(kqpad[:], 0, [[SC, NBLK], [1, WIN]], 2)
                        colK = rc.tile([128, NBLK * WIN], b16, tag="colK")
                        nc.vector.tensor_tensor(colK[:], colE[:], k_w, mult)

                        # rowE = 1/colE on in-block cols (e^{c-a} = 1/e^{a-c})
                        rowE = rc.tile([128, C], b16, tag="rowE")
                        inblk = pat(colE[:], WIN - SC, [[WIN, NBLK], [1, SC]], 2)
                        with nc.allow_low_precision(reason="bf16 rowE"):
                            nc.vector.reciprocal(rowE[:], inblk)
                        rowKQ = rc.tile([128, NBLK * 2 * SC], b16, tag="rowKQ")
                        dst_k = pat(rowKQ[:], 0, [[2 * SC, NBLK], [1, SC]], 2)
                        dst_q = pat(rowKQ[:], SC, [[2 * SC, NBLK], [1, SC]], 2)
                        src4 = lambda ap: pat(ap, 0, [[SC, NBLK], [1, SC]], 2)
                        nc.vector.tensor_tensor(dst_k, k_cm[:], src4(rowE[:]), mult)
                        nc.vector.tensor_tensor(dst_q, q_cm[:], src4(rowE[:]), mult)

                        # W/P row-block matmuls, 4 groups of 8 blocks
                        whalo = rc.tile([128, 192], b16, tag="whalo")
                        phalo = rc.tile([128, 192], b16, tag="phalo")
                        nc.gpsimd.memset(whalo[:], 0.0)
                        nc.gpsimd.memset(phalo[:], 0.0)
                        stage = rc.tile([8, 2048], b16, tag="wpstage")
                        for grp in range(4):
                            wp_ps = wpps.tile([128, 512], f32, tag="wp_ps")
                            for Ii in range(8):
                                I = grp * 8 + Ii
                                nc.tensor.matmul(
                                    wp_ps[0:2 * SC, Ii * WIN:(Ii + 1) * WIN],
                                    rowKQ[:, I * 2 * SC:(I + 1) * 2 * SC],
                                    colK[:, I * WIN:(I + 1) * WIN],
                                    start=True, stop=True,
                                )
                            nc.scalar.copy(
                                stage[0:8, grp * 512:(grp + 1) * 512], wp_ps[0:8, :]
                            )
                        # banded scatter via DRAM (flat addressing)
                        psg = list(stage[:].ap)[0][0]
                        nc.sync.dma_start(wdn[s], zro[:])
                        nc.sync.dma_start(pdn[s], zro[:])
                        for mat, dnt in ((0, wdn), (1, pdn)):
                            dst = dnt[s].copy()
                            dst.ap = bass_rust.VecI64Pair(
                                [[192, 4], [4 * 192 + 4, 32], [1, 64]])
                            srcb = pat(stage[:], mat * 4 * psg, [], 2)
                            srcb.ap = bass_rust.VecI64Pair(
                                [[psg, 4], [64, 32], [1, 64]])
                            nc.sync.dma_start(dst, srcb)
                        nc.sync.dma_start(whalo[:], wdn[s])
                        nc.sync.dma_start(phalo[:], pdn[s])
                        W_sb = rc.tile([128, C], b16, tag="W_sb")
                        P_sb = rc.tile([128, C], b16, tag="P_sb")
                        nc.vector.tensor_mul(W_sb[:], whalo[:, 60:188], mS_sb[:])
                        nc.vector.tensor_mul(P_sb[:], phalo[:, 60:188], mI_sb[:])

                        # A^T = -diag(beta) W^T ; A = transpose(A^T)
                        wtp = rcps_b.tile([128, C], b16, tag="psb")
                        nc.tensor.transpose(wtp[:], W_sb[:], idb_sb[:])
                        At = rc.tile([128, C], b16, tag="At")
                        nc.vector.tensor_scalar_mul(At[:], wtp[:], bcols[:, 1:2])
                        atp = rcps_b.tile([128, C], b16, tag="psb")
                        nc.tensor.transpose(atp[:], At[:], idb_sb[:])
                        A_sb = rc.tile([128, C], b16, tag="A_sb")
                        nc.vector.tensor_copy(A_sb[:], atp[:])
                        ptp = rcps_b.tile([128, C], b16, tag="psb")
                        nc.tensor.transpose(ptp[:], P_sb[:], idb_sb[:])
                        Pt = rc.tile([128, C], b16, tag="Pt")
                        nc.vector.tensor_copy(Pt[:], ptp[:])

                        # E128 = exp(c); Ktil/Qtil; Ehat/Khat; LamC
                        E128 = rc.tile([128, C], b16, tag="E128")
                        nc.scalar.activation(E128[:], c_cm, AF.Exp)
                        Ktil = rc.tile([128, C], b16, tag="Ktil")
                        nc.vector.tensor_mul(Ktil[:], k_cm[:], E128[:])
                        Qtil = rc.tile([128, C], b16, tag="Qtil")
                        nc.vector.tensor_mul(Qtil[:], q_cm[:], E128[:])
                        Ehat = rc.tile([128, C], b16, tag="Ehat")
                        nc.scalar.activation(
                            Ehat[:], c_cm, AF.Exp,
                            bias=c_cm[:, C - 1:C], scale=-1.0,
                        )
                        Khat = rc.tile([128, C], b16, tag="Khat")
                        nc.vector.tensor_mul(Khat[:], k_cm[:], Ehat[:])
                        khp = rcps_b.tile([128, C], b16, tag="psb")
                        nc.tensor.transpose(khp[:], Khat[:], idb_sb[:])
                        Khat_tm = rc.tile([128, C], b16, tag="Khat_tm")
                        nc.vector.tensor_copy(Khat_tm[:], khp[:])
                        lamc = rc.tile([128, 1], f32, tag="lamc")
                        nc.scalar.activation(lamc[:], c_cm[:, C - 1:C], AF.Exp)

                        # R = v - Ktil @ S
                        mem_ps = rcps_f.tile([128, C], f32, tag="psf")
                        nc.tensor.matmul(
                            mem_ps[:], Ktil[:], Sb[s][:], start=True, stop=True
                        )
                        Z = rc.tile([128, C], b16, tag="Zsolve")
                        nc.vector.tensor_tensor(Z[:], v_tm[:], mem_ps[:], sub)

                        # Neumann doubling: Z <- Z + Apow Z ; Apow <- Apow^2
                        Apow, ApowT = A_sb, At
                        for lev in range(KLEV + 1):
                            zp = rcps_f.tile([128, C], f32, tag="psf")
                            nc.scalar.copy(zp[:], Z[:])
                            nc.tensor.matmul(
                                zp[:], ApowT[:], Z[:], start=False, stop=True,
                                skip_group_check=True,
                            )
                            Z = rc.tile([128, C], b16, tag="Zsolve")
                            nc.vector.tensor_copy(Z[:], zp[:])
                            if lev < KLEV:
                                a2p = rcps_f.tile([128, C], f32, tag="psf")
                                nc.tensor.matmul(
                                    a2p[:], ApowT[:], Apow[:], start=True, stop=True
                                )
                                Apow = rc.tile([128, C], b16, tag="A_sb")
                                nc.vector.tensor_copy(Apow[:], a2p[:])
                                a2tp = rcps_b.tile([128, C], b16, tag="psb")
                                nc.tensor.transpose(a2tp[:], Apow[:], idb_sb[:])
                                ApowT = rc.tile([128, C], b16, tag="At")
                                nc.vector.tensor_copy(ApowT[:], a2tp[:])
                        U = rc.tile([128, C], b16, tag="U")
                        nc.vector.tensor_scalar_mul(U[:], Z[:], bcols[:, 0:1])

                        # o = Qtil^T S + P U
                        o_ps = rcps_f.tile([128, C], f32, tag="psf")
                        nc.tensor.matmul(o_ps[:], Qtil[:], Sb[s][:], start=True, stop=False)
                        nc.tensor.matmul(
                            o_ps[:], Pt[:], U[:], start=False, stop=True,
                            skip_group_check=True,
                        )
                        o_bf = rc.tile([128, C], b16, tag="o_bf")
                        nc.vector.tensor_copy(o_bf[:], o_ps[:])
                        nc.sync.dma_start(ot[csl, rsl], o_bf[:])

                        # S update: S = lamc*S + Khat^T U
                        sps = rcps_f.tile([128, C], f32, tag="psf")
                        nc.vector.tensor_scalar_mul(sps[:], Sf[s][:], lamc[:])
                        nc.tensor.matmul(
                            sps[:], Khat_tm[:], U[:], start=False, stop=True,
                            skip_group_check=True,
                        )
                        nc.vector.tensor_copy(Sf[s][:], sps[:])
                        nc.vector.tensor_copy(Sb[s][:], sps[:])

            # ============ Phase C2: RMS norm + sigmoid gate ============
            with (
                tc.tile_pool(name="c2", bufs=3) as c2p,
                tc.tile_pool(name="c2ps", bufs=3, space="PSUM") as c2ps,
            ):
                for r in range(32):
                    tsl = slice(r * 128, (r + 1) * 128)
                    o2 = c2p.tile([128, 256], b16, tag="o2")
                    nc.sync.dma_start(o2[:], ot[tsl, :])
                    for h in range(2):
                        hsl2 = slice(h * 128, (h + 1) * 128)
                        osq = c2p.tile([128, 128], b16, tag="osq")
                        ss = c2p.tile([128, 1], f32, tag="ss")
                        nc.scalar.activation(
                            osq[:], o2[:, hsl2], AF.Square, accum_out=ss[:]
                        )
                        rt2 = c2p.tile([128, 1], f32, tag="rt2")
                        nc.scalar.activation(
                            rt2[:], ss[:], AF.Sqrt, bias=epst[:, 0:1], scale=1.0 / D
                        )
                        rr2 = c2p.tile([128, 1], f32, tag="rr2")
                        nc.vector.reciprocal(rr2[:], rt2[:])
                        o_n = c2p.tile([128, 128], b16, tag="o_n")
                        nc.vector.tensor_scalar_mul(o_n[:], o2[:, hsl2], rr2[:])
                        onp = c2ps.tile([128, 128], b16, tag="onp")
                        nc.tensor.transpose(onp[:], o_n[:], idb_sb[:])
                        sgt = c2p.tile([128, 128], b16, tag="sgt")
                        nc.sync.dma_start(sgt[:], sg[hsl2, tsl])
                        ogt2 = c2p.tile([128, 128], b16, tag="ogt2")
                        nc.vector.tensor_mul(ogt2[:], onp[:], sgt[:])
                        nc.sync.dma_start(og[hsl2, tsl], ogt2[:])

            # ================= Phase D: output projection =================
            with (
                tc.tile_pool(name="op", bufs=4) as opool,
                tc.tile_pool(name="opps", bufs=4, space="PSUM") as oppool,
                tc.tile_pool(name="wop", bufs=1) as wopool,
            ):
                wo_t = []
                for c2 in range(2):
                    wot = wopool.tile([128, HID], b16, tag=f"wo{c2}")
                    nc.sync.dma_start(wot[:], wo[c2 * 128:(c2 + 1) * 128, :])
                    wo_t.append(wot)
                for tt in range(32):
                    ogt = []
                    for c2 in range(2):
                        t = opool.tile([128, 128], b16, tag="ogtile")
                        nc.sync.dma_start(
                            t[:], og[c2 * 128:(c2 + 1) * 128, tt * 128:(tt + 1) * 128]
                        )
                        ogt.append(t)
                    outsb = opool.tile([128, HID], b16, tag="outsb")
                    for n4 in range(4):
                        ps = oppool.tile([128, 512], f32, tag="ops")
                        for c2 in range(2):
                            nc.tensor.matmul(
                                ps[:], ogt[c2][:],
                                wo_t[c2][:, n4 * 512:(n4 + 1) * 512],
                                start=(c2 == 0), stop=(c2 == 1),
                            )
                        nc.scalar.copy(outsb[:, n4 * 512:(n4 + 1) * 512], ps[:])
                    nc.sync.dma_start(out[tt * 128:(tt + 1) * 128, :], outsb[:])

    nc.compile()
    return nc


def _chmaj(w):  # [256, 4] -> [128, 8] (ch-chunk along cols)
    return np.ascontiguousarray(w.reshape(2, 128, 4).transpose(1, 0, 2).reshape(128, 8)).astype(np.float32)


def _prep_inputs(inputs):
    f32 = np.float32
    hs = np.asarray(inputs["hidden_states"], f32).reshape(N, HID)
    hst = np.ascontiguousarray(hs.T).astype(bf16)       # [HID, N]

    def tile_w(w_t):  # [HID, M] -> [128, 16*M] kt-tiled
        m = w_t.shape[1]
        return np.ascontiguousarray(
            w_t.reshape(16, 128, m).transpose(1, 0, 2).reshape(128, 16 * m)
        ).astype(bf16)

    Wq, Wk, Wv = (np.asarray(inputs[x], f32) for x in ("Wq", "Wk", "Wv"))
    Wfa, Wfb = np.asarray(inputs["Wfa"], f32), np.asarray(inputs["Wfb"], f32)
    Wga, Wgb = np.asarray(inputs["Wga"], f32), np.asarray(inputs["Wgb"], f32)
    Wo = np.asarray(inputs["Wo"], f32)
    onw = np.asarray(inputs["o_norm_w"], f32)
    Wo_fold = Wo * np.tile(onw, H)[None, :]
    A = np.asarray(inputs["A_log"], f32).reshape(H)
    dt_bias = np.asarray(inputs["dt_bias"], f32)
    beta_all = 1.0 / (1.0 + np.exp(-(hs @ np.asarray(inputs["Wb"], f32).T)))  # [N, H]
    gpre = (hs @ Wfa.T) @ Wfb.T + dt_bias[None, :]
    aneg_full = np.repeat(-np.exp(A), D)[None, :]
    g_full = (aneg_full * np.logaddexp(0.0, gpre)).astype(f32)     # [N, P]

    Lx = np.triu(np.ones((128, 128), f32))              # L[j,t] = 1 if j <= t
    mS = np.tril(np.ones((128, 128), f32), -1).astype(bf16)
    mI = np.tril(np.ones((128, 128), f32), 0).astype(bf16)
    idb = np.eye(128, dtype=f32).astype(bf16)
    idf = np.eye(128, dtype=f32)
    pm1 = np.array([[1.0, -1.0]], f32)

    in_maps = []
    for core in range(8):
        cs = slice(core * 256, (core + 1) * 256)
        hsl = slice(core * 2, core * 2 + 2)
        im = {
            "hst": hst,
            "wq": tile_w(np.ascontiguousarray(Wq[cs].T)),
            "wk": tile_w(np.ascontiguousarray(Wk[cs].T)),
            "wv": tile_w(np.ascontiguousarray(Wv[cs].T)),
            "wga": tile_w(np.ascontiguousarray(Wga.T)),
            "wgb": np.ascontiguousarray(Wgb[cs].T).astype(bf16),
            "gs": np.ascontiguousarray(g_full[:, cs].T),
            "wo": np.ascontiguousarray(Wo_fold[:, cs].T).astype(bf16),
            "cwq": _chmaj(np.asarray(inputs["wq_conv"], f32)[cs]),
            "cwk": _chmaj(np.asarray(inputs["wk_conv"], f32)[cs]),
            "cwv": _chmaj(np.asarray(inputs["wv_conv"], f32)[cs]),
            "betah": np.ascontiguousarray(beta_all[:, hsl].T).astype(f32),
            "lx": Lx, "maskS": mS, "maskI": mI, "idb": idb, "idf": idf, "pm1": pm1,
        }
        in_maps.append(im)
    return in_maps


def _get_nc():
    if "nc" not in _CACHED:
        _CACHED["nc"] = _build_nc()
    return _CACHED["nc"]


def kernel(**inputs):
    from concourse.bass_utils import run_bass_kernel_spmd

    nc = _get_nc()
    in_maps = _prep_inputs(inputs)
    res = run_bass_kernel_spmd(nc, in_maps, list(range(8)))
    acc = np.zeros((N, HID), np.float32)
    for r in res.results:
        acc += np.asarray(r["out"], dtype=np.float32)
    return acc.reshape(B, T, HID)


# revision 26
# speedup vs baseline: 8.2829x; 8.2829x over previous
"""KimiDeltaAttention — Trainium2 Bass kernel, 8-core head-sharded SPMD.

Each core handles 2 of 16 heads (256 of 2048 channels):
  - q/k/v projections + causal depthwise conv + silu  (bf16 matmuls, fp32 psum)
  - decay gate g = -exp(A_log) * softplus(low-rank proj + dt_bias)   (fp32)
  - l2norm(q)*D^-0.5, l2norm(k)
  - chunked gated-delta-rule recurrence (chunk C=128, sub-chunk SC=4,
    block-start boundary factorization so every exp arg is <= +66; W/P via
    32 row-block matmuls; (I + W diag(beta))^{-1} via Neumann doubling)
  - gated RMSNorm (sigmoid low-rank gate), output projection partial
Host: shards inputs, computes beta (tiny), sums the 8 partial outputs.

Shapes hardcoded: B=2, T=2048, HID=2048, H=16, D=128, K=4.
"""

import os

os.environ.setdefault("JAX_COMPILATION_CACHE_DIR", "/root/jax_cache")
os.environ.setdefault("JAX_PERSISTENT_CACHE_MIN_ENTRY_SIZE_BYTES", "-1")
os.environ.setdefault("JAX_PERSISTENT_CACHE_MIN_COMPILE_TIME_SECS", "0")

import sys

if "/opt/trn_rl_repo" not in sys.path:
    sys.path.insert(0, "/opt/trn_rl_repo")

import numpy as np
import ml_dtypes

bf16 = ml_dtypes.bfloat16

B, T, HID = 2, 2048, 2048
H, D = 16, 128
P = H * D
N = B * T            # 4096 tokens
C = 128              # chunk length
SC = 4               # row-block
WIN = 64             # col window per row-block
NBLK = C // SC       # 32
NCH = T // C         # 16 chunks per sequence
EPS = 1e-6
KLEV = 4             # Neumann doubling levels (A^1..A^16)
BIGNEG_PAD = 3.0e38  # left-pad value so exp(a - pad) == 0

_CACHED = {}


def _config_jax_cache():
    try:
        import jax
        jax.config.update("jax_compilation_cache_dir", "/root/jax_cache")
        jax.config.update("jax_persistent_cache_min_entry_size_bytes", -1)
        jax.config.update("jax_persistent_cache_min_compile_time_secs", 0)
    except Exception:
        pass


def _build_nc():
    import concourse.bass as bass
    import concourse.tile as tile
    from concourse import bacc, mybir

    f32 = mybir.dt.float32
    b16 = mybir.dt.bfloat16

    nc = bacc.Bacc("TRN2", target_bir_lowering=False, debug=False, num_devices=8)

    # ---- I/O ----
    hst = nc.declare_dram_parameter("hst", [HID, N], b16, isOutput=False)          # hs^T tiled rows
    wq = nc.declare_dram_parameter("wq", [128, 16 * 256], b16, isOutput=False)     # kt-tiled W^T
    wk = nc.declare_dram_parameter("wk", [128, 16 * 256], b16, isOutput=False)
    wv = nc.declare_dram_parameter("wv", [128, 16 * 256], b16, isOutput=False)
    wga = nc.declare_dram_parameter("wga", [128, 16 * 128], b16, isOutput=False)
    wgb = nc.declare_dram_parameter("wgb", [128, 256], b16, isOutput=False)
    gs = nc.declare_dram_parameter("gs", [256, N], f32, isOutput=False)
    wo = nc.declare_dram_parameter("wo", [256, HID], b16, isOutput=False)          # rows=channels
    cwq = nc.declare_dram_parameter("cwq", [128, 8], f32, isOutput=False)
    cwk = nc.declare_dram_parameter("cwk", [128, 8], f32, isOutput=False)
    cwv = nc.declare_dram_parameter("cwv", [128, 8], f32, isOutput=False)
    betah = nc.declare_dram_parameter("betah", [2, N], f32, isOutput=False)
    lx = nc.declare_dram_parameter("lx", [128, 128], f32, isOutput=False)          # L[j,t]=1 if j<=t
    maskS = nc.declare_dram_parameter("maskS", [128, 128], b16, isOutput=False)    # strict lower
    maskI = nc.declare_dram_parameter("maskI", [128, 128], b16, isOutput=False)    # incl lower
    idb = nc.declare_dram_parameter("idb", [128, 128], b16, isOutput=False)        # identity bf16
    idf = nc.declare_dram_parameter("idf", [128, 128], f32, isOutput=False)        # identity f32
    pm1 = nc.declare_dram_parameter("pm1", [1, 2], f32, isOutput=False)            # [+1, -1]
    out = nc.declare_dram_parameter("out", [N, HID], b16, isOutput=True)

    # DRAM scratch
    qs = nc.dram_tensor("qs", [256, N], b16)
    ks = nc.dram_tensor("ks", [256, N], b16)
    vs = nc.dram_tensor("vs", [256, N], b16)
    ot = nc.dram_tensor("ot", [N, 256], b16)
    wdn = nc.dram_tensor("wdn", [4, 128, 192], b16)
    pdn = nc.dram_tensor("pdn", [4, 128, 192], b16)
    sg = nc.dram_tensor("sg", [256, N], b16)
    og = nc.dram_tensor("og", [256, N], b16)

    import bass_rust

    def pat(ap, offset_elems, dims, dtype_bytes):
        """Custom free-dim pattern on a 2D sbuf tile ap (keeps partition dim)."""
        c = ap.copy()
        part = list(c.ap)[0]
        c.ap = bass_rust.VecI64Pair([list(part)] + [list(d) for d in dims])
        c.offset = ap.offset + offset_elems
        return c

    with tile.TileContext(nc) as tc:
        with tc.tile_pool(name="consts", bufs=1) as cpool:
            lx_sb = cpool.tile([128, 128], f32)
            nc.sync.dma_start(lx_sb[:], lx[:])
            mS_sb = cpool.tile([128, 128], b16)
            nc.sync.dma_start(mS_sb[:], maskS[:])
            mI_sb = cpool.tile([128, 128], b16)
            nc.sync.dma_start(mI_sb[:], maskI[:])
            idb_sb = cpool.tile([128, 128], b16)
            nc.sync.dma_start(idb_sb[:], idb[:])
            idf_sb = cpool.tile([128, 128], f32)
            nc.sync.dma_start(idf_sb[:], idf[:])
            pm1_sb = cpool.tile([1, 2], f32)
            nc.sync.dma_start(pm1_sb[:], pm1[:])
            beta0 = cpool.tile([1, N], f32)
            nc.sync.dma_start(beta0[:], betah[0:1, :])
            beta1 = cpool.tile([1, N], f32)
            nc.sync.dma_start(beta1[:], betah[1:2, :])
            beta_t = (beta0, beta1)
            zro = cpool.tile([128, 192], b16)
            nc.gpsimd.memset(zro[:], 0.0)
            epst = cpool.tile([128, 2], f32)
            nc.gpsimd.memset(epst[:, 0:1], EPS)
            nc.gpsimd.memset(epst[:, 1:2], EPS * D)
            cw_sb = {}
            for nm, t in (("q", cwq), ("k", cwk), ("v", cwv)):
                cwt = cpool.tile([128, 8], f32, tag=f"cw{nm}")
                cw_sb[nm] = cwt
                nc.sync.dma_start(cw_sb[nm][:], t[:])

            # ================= Phase B: projections =================
            SEG = 2048 + 3
            with (
                tc.tile_pool(name="hts", bufs=1) as hpool,
                tc.tile_pool(name="wts", bufs=1) as wpool,
                tc.tile_pool(name="xp", bufs=1) as xpool,
                tc.tile_pool(name="proj", bufs=1) as ppool,
                tc.tile_pool(name="nrm", bufs=1) as npool,
                tc.tile_pool(name="projps", bufs=2, space="PSUM") as pps,
            ):
                w_sb = {}
                for nm, t in (("q", wq), ("k", wk), ("v", wv)):
                    wt_ = wpool.tile([128, 16 * 256], b16, tag=f"w{nm}")
                    w_sb[nm] = wt_
                    nc.sync.dma_start(w_sb[nm][:], t[:])
                for nm, t in (("ga", wga),):
                    wt_ = wpool.tile([128, 16 * 128], b16, tag=f"w{nm}")
                    w_sb[nm] = wt_
                    nc.sync.dma_start(w_sb[nm][:], t[:])
                for nm, t in (("gb", wgb),):
                    wt_ = wpool.tile([128, 256], b16, tag=f"w{nm}")
                    w_sb[nm] = wt_
                    nc.sync.dma_start(w_sb[nm][:], t[:])
                ones_col = wpool.tile([128, 1], b16, tag="ones")
                nc.gpsimd.memset(ones_col[:], 1.0)
                ones_row = wpool.tile([1, 128], b16, tag="onesr")
                nc.gpsimd.memset(ones_row[:], 1.0)

                xpads = {}
                for nm in ("q", "k", "v"):
                    for ch in range(2):
                        xp = xpool.tile([128, 2 * SEG], b16, tag=f"xp{nm}{ch}")
                        nc.gpsimd.memset(xp[:, 0:3], 0.0)
                        nc.gpsimd.memset(xp[:, SEG:SEG + 3], 0.0)
                        xpads[(nm, ch)] = xp
                lowr = {}
                for nm in ("ga",):
                    lrt = xpool.tile([128, N], b16, tag=f"lr{nm}")
                    lowr[nm] = lrt

                for nt in range(8):
                    hts = []
                    for kt in range(16):
                        ht = hpool.tile([128, 512], b16, tag=f"ht{kt}")
                        nc.sync.dma_start(
                            ht[:], hst[kt * 128:(kt + 1) * 128, nt * 512:(nt + 1) * 512]
                        )
                        hts.append(ht)
                    for nm in ("q", "k", "v"):
                        for ch in range(2):
                            ps = pps.tile([128, 512], f32, tag="pp")
                            for kt in range(16):
                                nc.tensor.matmul(
                                    ps[:],
                                    w_sb[nm][:, kt * 256 + ch * 128: kt * 256 + (ch + 1) * 128],
                                    hts[kt][:],
                                    start=(kt == 0), stop=(kt == 15),
                                )
                            b = nt // 4
                            col = b * SEG + 3 + (nt % 4) * 512
                            nc.scalar.copy(xpads[(nm, ch)][:, col:col + 512], ps[:])
                    for nm in ("ga",):
                        ps = pps.tile([128, 512], f32, tag="pp")
                        for kt in range(16):
                            nc.tensor.matmul(
                                ps[:], w_sb[nm][:, kt * 128:(kt + 1) * 128],
                                hts[kt][:], start=(kt == 0), stop=(kt == 15),
                            )
                        nc.scalar.copy(lowr[nm][:, nt * 512:(nt + 1) * 512], ps[:])

                # ---- gate: sg (bf16) ----
                for ch in range(2):
                    for nt in range(8):
                        ps2 = pps.tile([128, 512], f32, tag="pp")
                        nc.tensor.matmul(
                            ps2[:], w_sb["gb"][:, ch * 128:(ch + 1) * 128],
                            lowr["ga"][:, nt * 512:(nt + 1) * 512],
                            start=True, stop=True,
                        )
                        sgb = ppool.tile([128, 512], b16, tag="sgb")
                        nc.scalar.activation(
                            sgb[:], ps2[:], mybir.ActivationFunctionType.Sigmoid
                        )
                        nc.sync.dma_start(
                            sg[ch * 128:(ch + 1) * 128, nt * 512:(nt + 1) * 512], sgb[:]
                        )

                # ---- conv + silu (+ l2norm for q,k) ----
                mult = mybir.AluOpType.mult
                add = mybir.AluOpType.add
                for ch in range(2):
                    for nm, dst in (("q", qs), ("k", ks), ("v", vs)):
                        xpad = xpads[(nm, ch)]
                        wcol = cw_sb[nm][:, ch * 4:(ch + 1) * 4]
                        y = ppool.tile([128, N], b16, tag="ysb")
                        for b in range(2):
                            ysl = y[:, b * 2048:(b + 1) * 2048]
                            xb = xpad[:, b * SEG: (b + 1) * SEG]
                            nc.vector.tensor_scalar_mul(ysl, xb[:, 3:2051], wcol[:, 3:4])
                            for tau in (2, 1, 0):
                                nc.vector.scalar_tensor_tensor(
                                    ysl, xb[:, tau:tau + 2048], wcol[:, tau:tau + 1],
                                    ysl, mult, add,
                                )
                        ysil = ppool.tile([128, N], b16, tag="ysil")
                        nc.scalar.activation(
                            ysil[:], y[:], mybir.ActivationFunctionType.Silu
                        )
                        if nm == "v":
                            nc.sync.dma_start(dst[ch * 128:(ch + 1) * 128, :], ysil[:])
                            continue
                        # l2norm over d (partitions) via ones-matmul
                        ysq = ppool.tile([128, N], b16, tag="ysb")
                        nc.scalar.activation(
                            ysq[:], ysil[:], mybir.ActivationFunctionType.Square
                        )
                        ssq = npool.tile([1, N], f32, tag="ssq")
                        for nt in range(8):
                            ssp = pps.tile([1, 512], f32, tag="ssp")
                            nc.tensor.matmul(
                                ssp[:], ones_col[:],
                                ysq[:, nt * 512:(nt + 1) * 512],
                                start=True, stop=True,
                            )
                            nc.scalar.copy(ssq[:, nt * 512:(nt + 1) * 512], ssp[:])
                        scl = float(D) if nm == "q" else 1.0
                        bcol = epst[0:1, 1:2] if nm == "q" else epst[0:1, 0:1]
                        nc.scalar.activation(
                            ssq[:], ssq[:], mybir.ActivationFunctionType.Sqrt,
                            bias=bcol, scale=scl,
                        )
                        rr = npool.tile([1, N], b16, tag="rr")
                        with nc.allow_low_precision(reason="bf16 norm scale"):
                            nc.vector.reciprocal(rr[:], ssq[:])
                        yn = ppool.tile([128, N], b16, tag="ysb")
                        for nt in range(8):
                            sl = slice(nt * 512, (nt + 1) * 512)
                            bb = pps.tile([128, 512], f32, tag="pp")
                            nc.tensor.matmul(
                                bb[:], ones_row[:], rr[:, sl], start=True, stop=True
                            )
                            nc.vector.tensor_mul(yn[:, sl], ysil[:, sl], bb[:])
                        nc.sync.dma_start(dst[ch * 128:(ch + 1) * 128, :], yn[:])

            # ================= Phase C: recurrence =================
            mult = mybir.AluOpType.mult
            add = mybir.AluOpType.add
            sub = mybir.AluOpType.subtract
            AF = mybir.ActivationFunctionType
            with (
                tc.tile_pool(name="state", bufs=1) as spool,
                tc.tile_pool(name="rc", bufs=3) as rc,
                tc.tile_pool(name="rcpsf", bufs=3, space="PSUM") as rcps_f,
                tc.tile_pool(name="rcpsb", bufs=2, space="PSUM") as rcps_b,
                tc.tile_pool(name="wpps", bufs=2, space="PSUM") as wpps,
            ):
                Sf = []
                Sb = []
                for s in range(4):
                    sf = spool.tile([128, 128], f32, tag=f"Sf{s}")
                    nc.gpsimd.memset(sf[:], 0.0)
                    Sf.append(sf)
                    sbt = spool.tile([128, 128], b16, tag=f"Sb{s}")
                    nc.gpsimd.memset(sbt[:], 0.0)
                    Sb.append(sbt)

                for n in range(NCH):
                    for s in range(4):
                        b, h = s // 2, s % 2
                        t0 = b * T + n * C
                        rsl = slice(h * 128, (h + 1) * 128)
                        csl = slice(t0, t0 + C)

                        q_cm = rc.tile([128, C], b16, tag="q_cm")
                        nc.sync.dma_start(q_cm[:], qs[rsl, csl])
                        k_cm = rc.tile([128, C], b16, tag="k_cm")
                        nc.sync.dma_start(k_cm[:], ks[rsl, csl])
                        v_tm = rc.tile([128, C], b16, tag="v_tm")
                        nc.sync.dma_start_transpose(v_tm[:], vs[rsl, csl])
                        g_cm = rc.tile([128, C], f32, tag="g_cm")
                        nc.sync.dma_start(g_cm[:], gs[rsl, csl])
                        gtp = rcps_f.tile([128, C], f32, tag="psf")
                        nc.tensor.transpose(gtp[:], g_cm[:], idf_sb[:])
                        g_tm = rc.tile([128, C], f32, tag="g_tm")
                        nc.vector.tensor_copy(g_tm[:], gtp[:])

                        # beta columns: [128, 2] = (beta, -beta)
                        bps = rcps_f.tile([128, 2], f32, tag="psf")
                        nc.tensor.matmul(
                            bps[:], beta_t[h][:, csl], pm1_sb[:],
                            start=True, stop=True,
                        )
                        bcols = rc.tile([128, 2], f32, tag="bcols")
                        nc.scalar.copy(bcols[:], bps[:])

                        # cumsum c = L @ g_tm  -> c_tm -> transpose -> cpad
                        cps = rcps_f.tile([128, C], f32, tag="psf")
                        nc.tensor.matmul(cps[:], lx_sb[:], g_tm[:], start=True, stop=True)
                        c_tm = rc.tile([128, C], f32, tag="c_tm")
                        nc.scalar.copy(c_tm[:], cps[:])
                        ctp = rcps_f.tile([128, C], f32, tag="psf")
                        nc.tensor.transpose(ctp[:], c_tm[:], idf_sb[:])
                        cpad = rc.tile([128, 61 + C], f32, tag="cpad")
                        nc.gpsimd.memset(cpad[:, 0:60], BIGNEG_PAD)
                        nc.gpsimd.memset(cpad[:, 60:61], 0.0)
                        nc.vector.tensor_copy(cpad[:, 61:61 + C], ctp[:])
                        c_cm = cpad[:, 61:61 + C]

                        kqpad = rc.tile([128, 60 + C], b16, tag="kqpad")
                        nc.gpsimd.memset(kqpad[:, 0:60], 0.0)
                        nc.vector.tensor_copy(kqpad[:, 60:60 + C], k_cm[:])

                        # colD[d,(I,jj)] = c_j - a_I  (j = 4I-60+jj), 32 TS ops
                        colD = rc.tile([128, NBLK * WIN], f32, tag="colD")
                        for I in range(NBLK):
                            nc.vector.tensor_scalar_sub(
                                colD[:, I * WIN:(I + 1) * WIN],
                                cpad[:, 4 * I + 1:4 * I + 65],
                                cpad[:, 60 + 4 * I:61 + 4 * I],
                            )
                        colE = rc.tile([128, NBLK * WIN], b16, tag="colE")
                        nc.scalar.activation(colE[:], colD[:], AF.Exp, scale=-1.0)
                        k_w = pat(kqpad[:], 0, [[SC, NBLK], [1, WIN]], 2)
                        colK = rc.tile([128, NBLK * WIN], b16, tag="colK")
                        nc.vector.tensor_tensor(colK[:], colE[:], k_w, mult)

                        # rowE = 1/colE on in-block cols (e^{c-a} = 1/e^{a-c})
                        rowE = rc.tile([128, C], b16, tag="rowE")
                        inblk = pat(colE[:], WIN - SC, [[WIN, NBLK], [1, SC]], 2)
                        with nc.allow_low_precision(reason="bf16 rowE"):
                            nc.vector.reciprocal(rowE[:], inblk)
                        rowKQ = rc.tile([128, NBLK * 2 * SC], b16, tag="rowKQ")
                        dst_k = pat(rowKQ[:], 0, [[2 * SC, NBLK], [1, SC]], 2)
                        dst_q = pat(rowKQ[:], SC, [[2 * SC, NBLK], [1, SC]], 2)
                        src4 = lambda ap: pat(ap, 0, [[SC, NBLK], [1, SC]], 2)
                        nc.vector.tensor_tensor(dst_k, k_cm[:], src4(rowE[:]), mult)
                        nc.vector.tensor_tensor(dst_q, q_cm[:], src4(rowE[:]), mult)

                        # W/P row-block matmuls, 4 groups of 8 blocks
                        whalo = rc.tile([128, 192], b16, tag="whalo")
                        phalo = rc.tile([128, 192], b16, tag="phalo")
                        nc.gpsimd.memset(whalo[:], 0.0)
                        nc.gpsimd.memset(phalo[:], 0.0)
                        stage = rc.tile([8, 2048], b16, tag="wpstage")
                        for grp in range(4):
                            wp_ps = wpps.tile([128, 512], f32, tag="wp_ps")
                            for Ii in range(8):
                                I = grp * 8 + Ii
                                nc.tensor.matmul(
                                    wp_ps[0:2 * SC, Ii * WIN:(Ii + 1) * WIN],
                                    rowKQ[:, I * 2 * SC:(I + 1) * 2 * SC],
                                    colK[:, I * WIN:(I + 1) * WIN],
                                    start=True, stop=True,
                                )
                            nc.scalar.copy(
                                stage[0:8, grp * 512:(grp + 1) * 512], wp_ps[0:8, :]
                            )
                        # banded scatter via DRAM (flat addressing)
                        psg = list(stage[:].ap)[0][0]
                        nc.sync.dma_start(wdn[s], zro[:])
                        nc.sync.dma_start(pdn[s], zro[:])
                        for mat, dnt in ((0, wdn), (1, pdn)):
                            dst = dnt[s].copy()
                            dst.ap = bass_rust.VecI64Pair(
                                [[192, 4], [4 * 192 + 4, 32], [1, 64]])
                            srcb = pat(stage[:], mat * 4 * psg, [], 2)
                            srcb.ap = bass_rust.VecI64Pair(
                                [[psg, 4], [64, 32], [1, 64]])
                            nc.sync.dma_start(dst, srcb)
                        nc.sync.dma_start(whalo[:], wdn[s])
                        nc.sync.dma_start(phalo[:], pdn[s])
                        W_sb = rc.tile([128, C], b16, tag="W_sb")
                        P_sb = rc.tile([128, C], b16, tag="P_sb")
                        nc.vector.tensor_mul(W_sb[:], whalo[:, 60:188], mS_sb[:])
                        nc.vector.tensor_mul(P_sb[:], phalo[:, 60:188], mI_sb[:])

                        # A^T = -diag(beta) W^T ; A = transpose(A^T)
                        wtp = rcps_b.tile([128, C], b16, tag="psb")
                        nc.tensor.transpose(wtp[:], W_sb[:], idb_sb[:])
                        At = rc.tile([128, C], b16, tag="At")
                        nc.vector.tensor_scalar_mul(At[:], wtp[:], bcols[:, 1:2])
                        atp = rcps_b.tile([128, C], b16, tag="psb")
                        nc.tensor.transpose(atp[:], At[:], idb_sb[:])
                        A_sb = rc.tile([128, C], b16, tag="A_sb")
                        nc.vector.tensor_copy(A_sb[:], atp[:])
                        ptp = rcps_b.tile([128, C], b16, tag="psb")
                        nc.tensor.transpose(ptp[:], P_sb[:], idb_sb[:])
                        Pt = rc.tile([128, C], b16, tag="Pt")
                        nc.vector.tensor_copy(Pt[:], ptp[:])

                        # E128 = exp(c); Ktil/Qtil; Ehat/Khat; LamC
                        E128 = rc.tile([128, C], b16, tag="E128")
                        nc.scalar.activation(E128[:], c_cm, AF.Exp)
                        Ktil = rc.tile([128, C], b16, tag="Ktil")
                        nc.vector.tensor_mul(Ktil[:], k_cm[:], E128[:])
                        Qtil = rc.tile([128, C], b16, tag="Qtil")
                        nc.vector.tensor_mul(Qtil[:], q_cm[:], E128[:])
                        Ehat = rc.tile([128, C], b16, tag="Ehat")
                        nc.scalar.activation(
                            Ehat[:], c_cm, AF.Exp,
                            bias=c_cm[:, C - 1:C], scale=-1.0,
                        )
                        Khat = rc.tile([128, C], b16, tag="Khat")
                        nc.vector.tensor_mul(Khat[:], k_cm[:], Ehat[:])
                        khp = rcps_b.tile([128, C], b16, tag="psb")
                        nc.tensor.transpose(khp[:], Khat[:], idb_sb[:])
                        Khat_tm = rc.tile([128, C], b16, tag="Khat_tm")
                        nc.vector.tensor_copy(Khat_tm[:], khp[:])
                        lamc = rc.tile([128, 1], f32, tag="lamc")
                        nc.scalar.activation(lamc[:], c_cm[:, C - 1:C], AF.Exp)

                        # R = v - Ktil @ S
                        mem_ps = rcps_f.tile([128, C], f32, tag="psf")
                        nc.tensor.matmul(
                            mem_ps[:], Ktil[:], Sb[s][:], start=True, stop=True
                        )
                        Z = rc.tile([128, C], b16, tag="Zsolve")
                        nc.vector.tensor_tensor(Z[:], v_tm[:], mem_ps[:], sub)

                        # Neumann doubling: Z <- Z + Apow Z ; Apow <- Apow^2
                        Apow, ApowT = A_sb, At
                        for lev in range(KLEV + 1):
                            zp = rcps_f.tile([128, C], f32, tag="psf")
                            nc.scalar.copy(zp[:], Z[:])
                            nc.tensor.matmul(
                                zp[:], ApowT[:], Z[:], start=False, stop=True,
                                skip_group_check=True,
                            )
                            Z = rc.tile([128, C], b16, tag="Zsolve")
                            nc.vector.tensor_copy(Z[:], zp[:])
                            if lev < KLEV:
                                a2p = rcps_f.tile([128, C], f32, tag="psf")
                                nc.tensor.matmul(
                                    a2p[:], ApowT[:], Apow[:], start=True, stop=True
                                )
                                Apow = rc.tile([128, C], b16, tag="A_sb")
                                nc.vector.tensor_copy(Apow[:], a2p[:])
                                a2tp = rcps_b.tile([128, C], b16, tag="psb")
                                nc.tensor.transpose(a2tp[:], Apow[:], idb_sb[:])
                                ApowT = rc.tile([128, C], b16, tag="At")
                                nc.vector.tensor_copy(ApowT[:], a2tp[:])
                        U = rc.tile([128, C], b16, tag="U")
                        nc.vector.tensor_scalar_mul(U[:], Z[:], bcols[:, 0:1])

                        # o = Qtil^T S + P U
                        o_ps = rcps_f.tile([128, C], f32, tag="psf")
                        nc.tensor.matmul(o_ps[:], Qtil[:], Sb[s][:], start=True, stop=False)
                        nc.tensor.matmul(
                            o_ps[:], Pt[:], U[:], start=False, stop=True,
                            skip_group_check=True,
                        )
                        o_bf = rc.tile([128, C], b16, tag="o_bf")
                        nc.vector.tensor_copy(o_bf[:], o_ps[:])
                        nc.sync.dma_start(ot[csl, rsl], o_bf[:])

                        # S update: S = lamc*S + Khat^T U
                        sps = rcps_f.tile([128, C], f32, tag="psf")
                        nc.vector.tensor_scalar_mul(sps[:], Sf[s][:], lamc[:])
                        nc.tensor.matmul(
                            sps[:], Khat_tm[:], U[:], start=False, stop=True,
                            skip_group_check=True,
                        )
                        nc.vector.tensor_copy(Sf[s][:], sps[:])
                        nc.vector.tensor_copy(Sb[s][:], sps[:])

            # ============ Phase C2: RMS norm + sigmoid gate ============
            with (
                tc.tile_pool(name="c2", bufs=3) as c2p,
                tc.tile_pool(name="c2ps", bufs=3, space="PSUM") as c2ps,
            ):
                for r in range(32):
                    tsl = slice(r * 128, (r + 1) * 128)
                    o2 = c2p.tile([128, 256], b16, tag="o2")
                    nc.sync.dma_start(o2[:], ot[tsl, :])
                    for h in range(2):
                        hsl2 = slice(h * 128, (h + 1) * 128)
                        osq = c2p.tile([128, 128], b16, tag="osq")
                        ss = c2p.tile([128, 1], f32, tag="ss")
                        nc.scalar.activation(
                            osq[:], o2[:, hsl2], AF.Square, accum_out=ss[:]
                        )
                        rt2 = c2p.tile([128, 1], f32, tag="rt2")
                        nc.scalar.activation(
                            rt2[:], ss[:], AF.Sqrt, bias=epst[:, 0:1], scale=1.0 / D
                        )
                        rr2 = c2p.tile([128, 1], f32, tag="rr2")
                        nc.vector.reciprocal(rr2[:], rt2[:])
                        o_n = c2p.tile([128, 128], b16, tag="o_n")
                        nc.vector.tensor_scalar_mul(o_n[:], o2[:, hsl2], rr2[:])
                        onp = c2ps.tile([128, 128], b16, tag="onp")
                        nc.tensor.transpose(onp[:], o_n[:], idb_sb[:])
                        sgt = c2p.tile([128, 128], b16, tag="sgt")
                        nc.sync.dma_start(sgt[:], sg[hsl2, tsl])
                        ogt2 = c2p.tile([128, 128], b16, tag="ogt2")
                        nc.vector.tensor_mul(ogt2[:], onp[:], sgt[:])
                        nc.sync.dma_start(og[hsl2, tsl], ogt2[:])

            # ================= Phase D: output projection =================
            with (
                tc.tile_pool(name="op", bufs=4) as opool,
                tc.tile_pool(name="opps", bufs=4, space="PSUM") as oppool,
                tc.tile_pool(name="wop", bufs=1) as wopool,
            ):
                wo_t = []
                for c2 in range(2):
                    wot = wopool.tile([128, HID], b16, tag=f"wo{c2}")
                    nc.sync.dma_start(wot[:], wo[c2 * 128:(c2 + 1) * 128, :])
                    wo_t.append(wot)
                for tt in range(32):
                    ogt = []
                    for c2 in range(2):
                        t = opool.tile([128, 128], b16, tag="ogtile")
                        nc.sync.dma_start(
                            t[:], og[c2 * 128:(c2 + 1) * 128, tt * 128:(tt + 1) * 128]
                        )
                        ogt.append(t)
                    outsb = opool.tile([128, HID], b16, tag="outsb")
                    for n4 in range(4):
                        ps = oppool.tile([128, 512], f32, tag="ops")
                        for c2 in range(2):
                            nc.tensor.matmul(
                                ps[:], ogt[c2][:],
                                wo_t[c2][:, n4 * 512:(n4 + 1) * 512],
                                start=(c2 == 0), stop=(c2 == 1),
                            )
                        nc.scalar.copy(outsb[:, n4 * 512:(n4 + 1) * 512], ps[:])
                    nc.sync.dma_start(out[tt * 128:(tt + 1) * 128, :], outsb[:])

    nc.compile()
    return nc


def _chmaj(w):  # [256, 4] -> [128, 8] (ch-chunk along cols)
    return np.ascontiguousarray(w.reshape(2, 128, 4).transpose(1, 0, 2).reshape(128, 8)).astype(np.float32)


def _prep_inputs(inputs):
    f32 = np.float32
    hs = np.asarray(inputs["hidden_states"], f32).reshape(N, HID)
    hst = np.ascontiguousarray(hs.T).astype(bf16)       # [HID, N]

    def tile_w(w_t):  # [HID, M] -> [128, 16*M] kt-tiled
        m = w_t.shape[1]
        return np.ascontiguousarray(
            w_t.reshape(16, 128, m).transpose(1, 0, 2).reshape(128, 16 * m)
        ).astype(bf16)

    Wq, Wk, Wv = (np.asarray(inputs[x], f32) for x in ("Wq", "Wk", "Wv"))
    Wfa, Wfb = np.asarray(inputs["Wfa"], f32), np.asarray(inputs["Wfb"], f32)
    Wga, Wgb = np.asarray(inputs["Wga"], f32), np.asarray(inputs["Wgb"], f32)
    Wo = np.asarray(inputs["Wo"], f32)
    onw = np.asarray(inputs["o_norm_w"], f32)
    Wo_fold = Wo * np.tile(onw, H)[None, :]
    A = np.asarray(inputs["A_log"], f32).reshape(H)
    dt_bias = np.asarray(inputs["dt_bias"], f32)
    beta_all = 1.0 / (1.0 + np.exp(-(hs @ np.asarray(inputs["Wb"], f32).T)))  # [N, H]
    gpre = (hs @ Wfa.T) @ Wfb.T + dt_bias[None, :]
    aneg_full = np.repeat(-np.exp(A), D)[None, :]
    g_full = (aneg_full * np.logaddexp(0.0, gpre)).astype(f32)     # [N, P]

    Lx = np.triu(np.ones((128, 128), f32))              # L[j,t] = 1 if j <= t
    mS = np.tril(np.ones((128, 128), f32), -1).astype(bf16)
    mI = np.tril(np.ones((128, 128), f32), 0).astype(bf16)
    idb = np.eye(128, dtype=f32).astype(bf16)
    idf = np.eye(128, dtype=f32)
    pm1 = np.array([[1.0, -1.0]], f32)

    in_maps = []
    for core in range(8):
        cs = slice(core * 256, (core + 1) * 256)
        hsl = slice(core * 2, core * 2 + 2)
        im = {
            "hst": hst,
            "wq": tile_w(np.ascontiguousarray(Wq[cs].T)),
            "wk": tile_w(np.ascontiguousarray(Wk[cs].T)),
            "wv": tile_w(np.ascontiguousarray(Wv[cs].T)),
            "wga": tile_w(np.ascontiguousarray(Wga.T)),
            "wgb": np.ascontiguousarray(Wgb[cs].T).astype(bf16),
            "gs": np.ascontiguousarray(g_full[:, cs].T),
            "wo": np.ascontiguousarray(Wo_fold[:, cs].T).astype(bf16),
            "cwq": _chmaj(np.asarray(inputs["wq_conv"], f32)[cs]),
            "cwk": _chmaj(np.asarray(inputs["wk_conv"], f32)[cs]),
            "cwv": _chmaj(np.asarray(inputs["wv_conv"], f32)[cs]),
            "betah": np.ascontiguousarray(beta_all[:, hsl].T).astype(f32),
            "lx": Lx, "maskS": mS, "maskI": mI, "idb": idb, "idf": idf, "pm1": pm1,
        }
        in_maps.append(im)
    return in_maps


def _get_nc():
    if "nc" not in _CACHED:
        _CACHED["nc"] = _build_nc()
    return _CACHED["nc"]


def kernel(**inputs):
    _config_jax_cache()
    from concourse.bass_utils import run_bass_kernel_spmd

    nc = _get_nc()
    in_maps = _prep_inputs(inputs)
    res = run_bass_kernel_spmd(nc, in_maps, list(range(8)))
    acc = np.zeros((N, HID), np.float32)
    for r in res.results:
        acc += np.asarray(r["out"], dtype=np.float32)
    return acc.reshape(B, T, HID)


# revision 28
# speedup vs baseline: 17.3509x; 2.0948x over previous
"""KimiDeltaAttention — Trainium2 Bass kernel, 8-core head-sharded SPMD.

Each core handles 2 of 16 heads (256 of 2048 channels):
  - q/k/v projections + causal depthwise conv + silu  (bf16 matmuls, fp32 psum)
  - decay gate g = -exp(A_log) * softplus(low-rank proj + dt_bias)   (fp32)
  - l2norm(q)*D^-0.5, l2norm(k)
  - chunked gated-delta-rule recurrence (chunk C=128, sub-chunk SC=4,
    block-start boundary factorization so every exp arg is <= +66; W/P via
    32 row-block matmuls; (I + W diag(beta))^{-1} via Neumann doubling)
  - gated RMSNorm (sigmoid low-rank gate), output projection partial
Host: shards inputs, computes beta (tiny), sums the 8 partial outputs.

Shapes hardcoded: B=2, T=2048, HID=2048, H=16, D=128, K=4.
"""

import os

os.environ.setdefault("JAX_COMPILATION_CACHE_DIR", "/root/jax_cache")
os.environ.setdefault("JAX_PERSISTENT_CACHE_MIN_ENTRY_SIZE_BYTES", "-1")
os.environ.setdefault("JAX_PERSISTENT_CACHE_MIN_COMPILE_TIME_SECS", "0")

import sys

if "/opt/trn_rl_repo" not in sys.path:
    sys.path.insert(0, "/opt/trn_rl_repo")

import numpy as np
import ml_dtypes

bf16 = ml_dtypes.bfloat16

B, T, HID = 2, 2048, 2048
H, D = 16, 128
P = H * D
N = B * T            # 4096 tokens
C = 128              # chunk length
SC = 4               # row-block
WIN = 64             # col window per row-block
NBLK = C // SC       # 32
NCH = T // C         # 16 chunks per sequence
EPS = 1e-6
KLEV = 4             # Neumann doubling levels (A^1..A^16)
BIGNEG_PAD = 3.0e38  # left-pad value so exp(a - pad) == 0

_CACHED = {}


def _config_jax_cache():
    try:
        import jax
        jax.config.update("jax_compilation_cache_dir", "/root/jax_cache")
        jax.config.update("jax_persistent_cache_min_entry_size_bytes", -1)
        jax.config.update("jax_persistent_cache_min_compile_time_secs", 0)
    except Exception:
        pass


def _build_nc():
    import concourse.bass as bass
    import concourse.tile as tile
    from concourse import bacc, mybir

    f32 = mybir.dt.float32
    b16 = mybir.dt.bfloat16

    nc = bacc.Bacc("TRN2", target_bir_lowering=False, debug=False, num_devices=8)

    # ---- I/O ----
    hst = nc.declare_dram_parameter("hst", [HID, N], b16, isOutput=False)          # hs^T tiled rows
    wq = nc.declare_dram_parameter("wq", [128, 16 * 256], b16, isOutput=False)     # kt-tiled W^T
    wk = nc.declare_dram_parameter("wk", [128, 16 * 256], b16, isOutput=False)
    wv = nc.declare_dram_parameter("wv", [128, 16 * 256], b16, isOutput=False)
    wga = nc.declare_dram_parameter("wga", [128, 16 * 128], b16, isOutput=False)
    wgb = nc.declare_dram_parameter("wgb", [128, 256], b16, isOutput=False)
    gs = nc.declare_dram_parameter("gs", [256, N], f32, isOutput=False)
    wo = nc.declare_dram_parameter("wo", [256, HID], b16, isOutput=False)          # rows=channels
    cwq = nc.declare_dram_parameter("cwq", [128, 8], f32, isOutput=False)
    cwk = nc.declare_dram_parameter("cwk", [128, 8], f32, isOutput=False)
    cwv = nc.declare_dram_parameter("cwv", [128, 8], f32, isOutput=False)
    betah = nc.declare_dram_parameter("betah", [2, N], f32, isOutput=False)
    lx = nc.declare_dram_parameter("lx", [128, 128], f32, isOutput=False)          # L[j,t]=1 if j<=t
    maskS = nc.declare_dram_parameter("maskS", [128, 128], b16, isOutput=False)    # strict lower
    maskI = nc.declare_dram_parameter("maskI", [128, 128], b16, isOutput=False)    # incl lower
    idb = nc.declare_dram_parameter("idb", [128, 128], b16, isOutput=False)        # identity bf16
    idf = nc.declare_dram_parameter("idf", [128, 128], f32, isOutput=False)        # identity f32
    pm1 = nc.declare_dram_parameter("pm1", [1, 2], f32, isOutput=False)            # [+1, -1]
    out = nc.declare_dram_parameter("out", [N // 8, HID], b16, isOutput=True)

    # DRAM scratch
    qs = nc.dram_tensor("qs", [256, N], b16)
    ks = nc.dram_tensor("ks", [256, N], b16)
    vs = nc.dram_tensor("vs", [256, N], b16)
    ot = nc.dram_tensor("ot", [N, 256], b16)
    opf = nc.dram_tensor("opf", [N, HID], b16)
    rso = nc.dram_tensor("rso", [N // 8, HID], b16)
    wdn = nc.dram_tensor("wdn", [4, 128, 192], b16)
    pdn = nc.dram_tensor("pdn", [4, 128, 192], b16)
    sg = nc.dram_tensor("sg", [256, N], b16)
    og = nc.dram_tensor("og", [256, N], b16)

    import bass_rust

    def pat(ap, offset_elems, dims, dtype_bytes):
        """Custom free-dim pattern on a 2D sbuf tile ap (keeps partition dim)."""
        c = ap.copy()
        part = list(c.ap)[0]
        c.ap = bass_rust.VecI64Pair([list(part)] + [list(d) for d in dims])
        c.offset = ap.offset + offset_elems
        return c

    with tile.TileContext(nc) as tc:
        with tc.tile_pool(name="consts", bufs=1) as cpool:
            lx_sb = cpool.tile([128, 128], f32)
            nc.sync.dma_start(lx_sb[:], lx[:])
            mS_sb = cpool.tile([128, 128], b16)
            nc.sync.dma_start(mS_sb[:], maskS[:])
            mI_sb = cpool.tile([128, 128], b16)
            nc.sync.dma_start(mI_sb[:], maskI[:])
            idb_sb = cpool.tile([128, 128], b16)
            nc.sync.dma_start(idb_sb[:], idb[:])
            idf_sb = cpool.tile([128, 128], f32)
            nc.sync.dma_start(idf_sb[:], idf[:])
            pm1_sb = cpool.tile([1, 2], f32)
            nc.sync.dma_start(pm1_sb[:], pm1[:])
            beta0 = cpool.tile([1, N], f32)
            nc.sync.dma_start(beta0[:], betah[0:1, :])
            beta1 = cpool.tile([1, N], f32)
            nc.sync.dma_start(beta1[:], betah[1:2, :])
            beta_t = (beta0, beta1)
            zro = cpool.tile([128, 192], b16)
            nc.gpsimd.memset(zro[:], 0.0)
            epst = cpool.tile([128, 2], f32)
            nc.gpsimd.memset(epst[:, 0:1], EPS)
            nc.gpsimd.memset(epst[:, 1:2], EPS * D)
            cw_sb = {}
            for nm, t in (("q", cwq), ("k", cwk), ("v", cwv)):
                cwt = cpool.tile([128, 8], f32, tag=f"cw{nm}")
                cw_sb[nm] = cwt
                nc.sync.dma_start(cw_sb[nm][:], t[:])

            # ================= Phase B: projections =================
            SEG = 2048 + 3
            with (
                tc.tile_pool(name="hts", bufs=1) as hpool,
                tc.tile_pool(name="wts", bufs=1) as wpool,
                tc.tile_pool(name="xp", bufs=1) as xpool,
                tc.tile_pool(name="proj", bufs=1) as ppool,
                tc.tile_pool(name="nrm", bufs=1) as npool,
                tc.tile_pool(name="projps", bufs=2, space="PSUM") as pps,
            ):
                w_sb = {}
                for nm, t in (("q", wq), ("k", wk), ("v", wv)):
                    wt_ = wpool.tile([128, 16 * 256], b16, tag=f"w{nm}")
                    w_sb[nm] = wt_
                    nc.sync.dma_start(w_sb[nm][:], t[:])
                for nm, t in (("ga", wga),):
                    wt_ = wpool.tile([128, 16 * 128], b16, tag=f"w{nm}")
                    w_sb[nm] = wt_
                    nc.sync.dma_start(w_sb[nm][:], t[:])
                for nm, t in (("gb", wgb),):
                    wt_ = wpool.tile([128, 256], b16, tag=f"w{nm}")
                    w_sb[nm] = wt_
                    nc.sync.dma_start(w_sb[nm][:], t[:])
                ones_col = wpool.tile([128, 1], b16, tag="ones")
                nc.gpsimd.memset(ones_col[:], 1.0)
                ones_row = wpool.tile([1, 128], b16, tag="onesr")
                nc.gpsimd.memset(ones_row[:], 1.0)

                xpads = {}
                for nm in ("q", "k", "v"):
                    for ch in range(2):
                        xp = xpool.tile([128, 2 * SEG], b16, tag=f"xp{nm}{ch}")
                        nc.gpsimd.memset(xp[:, 0:3], 0.0)
                        nc.gpsimd.memset(xp[:, SEG:SEG + 3], 0.0)
                        xpads[(nm, ch)] = xp
                lowr = {}
                for nm in ("ga",):
                    lrt = xpool.tile([128, N], b16, tag=f"lr{nm}")
                    lowr[nm] = lrt

                for nt in range(8):
                    hts = []
                    for kt in range(16):
                        ht = hpool.tile([128, 512], b16, tag=f"ht{kt}")
                        nc.sync.dma_start(
                            ht[:], hst[kt * 128:(kt + 1) * 128, nt * 512:(nt + 1) * 512]
                        )
                        hts.append(ht)
                    for nm in ("q", "k", "v"):
                        for ch in range(2):
                            ps = pps.tile([128, 512], f32, tag="pp")
                            for kt in range(16):
                                nc.tensor.matmul(
                                    ps[:],
                                    w_sb[nm][:, kt * 256 + ch * 128: kt * 256 + (ch + 1) * 128],
                                    hts[kt][:],
                                    start=(kt == 0), stop=(kt == 15),
                                )
                            b = nt // 4
                            col = b * SEG + 3 + (nt % 4) * 512
                            nc.scalar.copy(xpads[(nm, ch)][:, col:col + 512], ps[:])
                    for nm in ("ga",):
                        ps = pps.tile([128, 512], f32, tag="pp")
                        for kt in range(16):
                            nc.tensor.matmul(
                                ps[:], w_sb[nm][:, kt * 128:(kt + 1) * 128],
                                hts[kt][:], start=(kt == 0), stop=(kt == 15),
                            )
                        nc.scalar.copy(lowr[nm][:, nt * 512:(nt + 1) * 512], ps[:])

                # ---- gate: sg (bf16) ----
                for ch in range(2):
                    for nt in range(8):
                        ps2 = pps.tile([128, 512], f32, tag="pp")
                        nc.tensor.matmul(
                            ps2[:], w_sb["gb"][:, ch * 128:(ch + 1) * 128],
                            lowr["ga"][:, nt * 512:(nt + 1) * 512],
                            start=True, stop=True,
                        )
                        sgb = ppool.tile([128, 512], b16, tag="sgb")
                        nc.scalar.activation(
                            sgb[:], ps2[:], mybir.ActivationFunctionType.Sigmoid
                        )
                        nc.sync.dma_start(
                            sg[ch * 128:(ch + 1) * 128, nt * 512:(nt + 1) * 512], sgb[:]
                        )

                # ---- conv + silu (+ l2norm for q,k) ----
                mult = mybir.AluOpType.mult
                add = mybir.AluOpType.add
                for ch in range(2):
                    for nm, dst in (("q", qs), ("k", ks), ("v", vs)):
                        xpad = xpads[(nm, ch)]
                        wcol = cw_sb[nm][:, ch * 4:(ch + 1) * 4]
                        y = ppool.tile([128, N], b16, tag="ysb")
                        for b in range(2):
                            ysl = y[:, b * 2048:(b + 1) * 2048]
                            xb = xpad[:, b * SEG: (b + 1) * SEG]
                            nc.vector.tensor_scalar_mul(ysl, xb[:, 3:2051], wcol[:, 3:4])
                            for tau in (2, 1, 0):
                                nc.vector.scalar_tensor_tensor(
                                    ysl, xb[:, tau:tau + 2048], wcol[:, tau:tau + 1],
                                    ysl, mult, add,
                                )
                        ysil = ppool.tile([128, N], b16, tag="ysil")
                        nc.scalar.activation(
                            ysil[:], y[:], mybir.ActivationFunctionType.Silu
                        )
                        if nm == "v":
                            nc.sync.dma_start(dst[ch * 128:(ch + 1) * 128, :], ysil[:])
                            continue
                        # l2norm over d (partitions) via ones-matmul
                        ysq = ppool.tile([128, N], b16, tag="ysb")
                        nc.scalar.activation(
                            ysq[:], ysil[:], mybir.ActivationFunctionType.Square
                        )
                        ssq = npool.tile([1, N], f32, tag="ssq")
                        for nt in range(8):
                            ssp = pps.tile([1, 512], f32, tag="ssp")
                            nc.tensor.matmul(
                                ssp[:], ones_col[:],
                                ysq[:, nt * 512:(nt + 1) * 512],
                                start=True, stop=True,
                            )
                            nc.scalar.copy(ssq[:, nt * 512:(nt + 1) * 512], ssp[:])
                        scl = float(D) if nm == "q" else 1.0
                        bcol = epst[0:1, 1:2] if nm == "q" else epst[0:1, 0:1]
                        nc.scalar.activation(
                            ssq[:], ssq[:], mybir.ActivationFunctionType.Sqrt,
                            bias=bcol, scale=scl,
                        )
                        rr = npool.tile([1, N], b16, tag="rr")
                        with nc.allow_low_precision(reason="bf16 norm scale"):
                            nc.vector.reciprocal(rr[:], ssq[:])
                        yn = ppool.tile([128, N], b16, tag="ysb")
                        for nt in range(8):
                            sl = slice(nt * 512, (nt + 1) * 512)
                            bb = pps.tile([128, 512], f32, tag="pp")
                            nc.tensor.matmul(
                                bb[:], ones_row[:], rr[:, sl], start=True, stop=True
                            )
                            nc.vector.tensor_mul(yn[:, sl], ysil[:, sl], bb[:])
                        nc.sync.dma_start(dst[ch * 128:(ch + 1) * 128, :], yn[:])

            # ================= Phase C: recurrence =================
            mult = mybir.AluOpType.mult
            add = mybir.AluOpType.add
            sub = mybir.AluOpType.subtract
            AF = mybir.ActivationFunctionType
            with (
                tc.tile_pool(name="state", bufs=1) as spool,
                tc.tile_pool(name="rc", bufs=3) as rc,
                tc.tile_pool(name="rcpsf", bufs=3, space="PSUM") as rcps_f,
                tc.tile_pool(name="rcpsb", bufs=2, space="PSUM") as rcps_b,
                tc.tile_pool(name="wpps", bufs=2, space="PSUM") as wpps,
            ):
                Sf = []
                Sb = []
                for s in range(4):
                    sf = spool.tile([128, 128], f32, tag=f"Sf{s}")
                    nc.gpsimd.memset(sf[:], 0.0)
                    Sf.append(sf)
                    sbt = spool.tile([128, 128], b16, tag=f"Sb{s}")
                    nc.gpsimd.memset(sbt[:], 0.0)
                    Sb.append(sbt)

                for n in range(NCH):
                    for s in range(4):
                        b, h = s // 2, s % 2
                        t0 = b * T + n * C
                        rsl = slice(h * 128, (h + 1) * 128)
                        csl = slice(t0, t0 + C)

                        q_cm = rc.tile([128, C], b16, tag="q_cm")
                        nc.sync.dma_start(q_cm[:], qs[rsl, csl])
                        k_cm = rc.tile([128, C], b16, tag="k_cm")
                        nc.sync.dma_start(k_cm[:], ks[rsl, csl])
                        v_tm = rc.tile([128, C], b16, tag="v_tm")
                        nc.sync.dma_start_transpose(v_tm[:], vs[rsl, csl])
                        g_cm = rc.tile([128, C], f32, tag="g_cm")
                        nc.sync.dma_start(g_cm[:], gs[rsl, csl])
                        gtp = rcps_f.tile([128, C], f32, tag="psf")
                        nc.tensor.transpose(gtp[:], g_cm[:], idf_sb[:])
                        g_tm = rc.tile([128, C], f32, tag="g_tm")
                        nc.vector.tensor_copy(g_tm[:], gtp[:])

                        # beta columns: [128, 2] = (beta, -beta)
                        bps = rcps_f.tile([128, 2], f32, tag="psf")
                        nc.tensor.matmul(
                            bps[:], beta_t[h][:, csl], pm1_sb[:],
                            start=True, stop=True,
                        )
                        bcols = rc.tile([128, 2], f32, tag="bcols")
                        nc.scalar.copy(bcols[:], bps[:])

                        # cumsum c = L @ g_tm  -> c_tm -> transpose -> cpad
                        cps = rcps_f.tile([128, C], f32, tag="psf")
                        nc.tensor.matmul(cps[:], lx_sb[:], g_tm[:], start=True, stop=True)
                        c_tm = rc.tile([128, C], f32, tag="c_tm")
                        nc.scalar.copy(c_tm[:], cps[:])
                        ctp = rcps_f.tile([128, C], f32, tag="psf")
                        nc.tensor.transpose(ctp[:], c_tm[:], idf_sb[:])
                        cpad = rc.tile([128, 61 + C], f32, tag="cpad")
                        nc.gpsimd.memset(cpad[:, 0:60], BIGNEG_PAD)
                        nc.gpsimd.memset(cpad[:, 60:61], 0.0)
                        nc.vector.tensor_copy(cpad[:, 61:61 + C], ctp[:])
                        c_cm = cpad[:, 61:61 + C]

                        kqpad = rc.tile([128, 60 + C], b16, tag="kqpad")
                        nc.gpsimd.memset(kqpad[:, 0:60], 0.0)
                        nc.vector.tensor_copy(kqpad[:, 60:60 + C], k_cm[:])

                        # colD[d,(I,jj)] = c_j - a_I  (j = 4I-60+jj), 32 TS ops
                        colD = rc.tile([128, NBLK * WIN], f32, tag="colD")
                        for I in range(NBLK):
                            nc.vector.tensor_scalar_sub(
                                colD[:, I * WIN:(I + 1) * WIN],
                                cpad[:, 4 * I + 1:4 * I + 65],
                                cpad[:, 60 + 4 * I:61 + 4 * I],
                            )
                        colE = rc.tile([128, NBLK * WIN], b16, tag="colE")
                        nc.scalar.activation(colE[:], colD[:], AF.Exp, scale=-1.0)
                        k_w = pat(kqpad[:], 0, [[SC, NBLK], [1, WIN]], 2)
                        colK = rc.tile([128, NBLK * WIN], b16, tag="colK")
                        nc.vector.tensor_tensor(colK[:], colE[:], k_w, mult)

                        # rowE = 1/colE on in-block cols (e^{c-a} = 1/e^{a-c})
                        rowE = rc.tile([128, C], b16, tag="rowE")
                        inblk = pat(colE[:], WIN - SC, [[WIN, NBLK], [1, SC]], 2)
                        with nc.allow_low_precision(reason="bf16 rowE"):
                            nc.vector.reciprocal(rowE[:], inblk)
                        rowKQ = rc.tile([128, NBLK * 2 * SC], b16, tag="rowKQ")
                        dst_k = pat(rowKQ[:], 0, [[2 * SC, NBLK], [1, SC]], 2)
                        dst_q = pat(rowKQ[:], SC, [[2 * SC, NBLK], [1, SC]], 2)
                        src4 = lambda ap: pat(ap, 0, [[SC, NBLK], [1, SC]], 2)
                        nc.vector.tensor_tensor(dst_k, k_cm[:], src4(rowE[:]), mult)
                        nc.vector.tensor_tensor(dst_q, q_cm[:], src4(rowE[:]), mult)

                        # W/P row-block matmuls, 4 groups of 8 blocks
                        whalo = rc.tile([128, 192], b16, tag="whalo")
                        phalo = rc.tile([128, 192], b16, tag="phalo")
                        nc.gpsimd.memset(whalo[:], 0.0)
                        nc.gpsimd.memset(phalo[:], 0.0)
                        stage = rc.tile([8, 2048], b16, tag="wpstage")
                        for grp in range(4):
                            wp_ps = wpps.tile([128, 512], f32, tag="wp_ps")
                            for Ii in range(8):
                                I = grp * 8 + Ii
                                nc.tensor.matmul(
                                    wp_ps[0:2 * SC, Ii * WIN:(Ii + 1) * WIN],
                                    rowKQ[:, I * 2 * SC:(I + 1) * 2 * SC],
                                    colK[:, I * WIN:(I + 1) * WIN],
                                    start=True, stop=True,
                                )
                            nc.scalar.copy(
                                stage[0:8, grp * 512:(grp + 1) * 512], wp_ps[0:8, :]
                            )
                        # banded scatter via DRAM (flat addressing)
                        psg = list(stage[:].ap)[0][0]
                        nc.sync.dma_start(wdn[s], zro[:])
                        nc.sync.dma_start(pdn[s], zro[:])
                        for mat, dnt in ((0, wdn), (1, pdn)):
                            dst = dnt[s].copy()
                            dst.ap = bass_rust.VecI64Pair(
                                [[192, 4], [4 * 192 + 4, 32], [1, 64]])
                            srcb = pat(stage[:], mat * 4 * psg, [], 2)
                            srcb.ap = bass_rust.VecI64Pair(
                                [[psg, 4], [64, 32], [1, 64]])
                            nc.sync.dma_start(dst, srcb)
                        nc.sync.dma_start(whalo[:], wdn[s])
                        nc.sync.dma_start(phalo[:], pdn[s])
                        W_sb = rc.tile([128, C], b16, tag="W_sb")
                        P_sb = rc.tile([128, C], b16, tag="P_sb")
                        nc.vector.tensor_mul(W_sb[:], whalo[:, 60:188], mS_sb[:])
                        nc.vector.tensor_mul(P_sb[:], phalo[:, 60:188], mI_sb[:])

                        # A^T = -diag(beta) W^T ; A = transpose(A^T)
                        wtp = rcps_b.tile([128, C], b16, tag="psb")
                        nc.tensor.transpose(wtp[:], W_sb[:], idb_sb[:])
                        At = rc.tile([128, C], b16, tag="At")
                        nc.vector.tensor_scalar_mul(At[:], wtp[:], bcols[:, 1:2])
                        atp = rcps_b.tile([128, C], b16, tag="psb")
                        nc.tensor.transpose(atp[:], At[:], idb_sb[:])
                        A_sb = rc.tile([128, C], b16, tag="A_sb")
                        nc.vector.tensor_copy(A_sb[:], atp[:])
                        ptp = rcps_b.tile([128, C], b16, tag="psb")
                        nc.tensor.transpose(ptp[:], P_sb[:], idb_sb[:])
                        Pt = rc.tile([128, C], b16, tag="Pt")
                        nc.vector.tensor_copy(Pt[:], ptp[:])

                        # E128 = exp(c); Ktil/Qtil; Ehat/Khat; LamC
                        E128 = rc.tile([128, C], b16, tag="E128")
                        nc.scalar.activation(E128[:], c_cm, AF.Exp)
                        Ktil = rc.tile([128, C], b16, tag="Ktil")
                        nc.vector.tensor_mul(Ktil[:], k_cm[:], E128[:])
                        Qtil = rc.tile([128, C], b16, tag="Qtil")
                        nc.vector.tensor_mul(Qtil[:], q_cm[:], E128[:])
                        Ehat = rc.tile([128, C], b16, tag="Ehat")
                        nc.scalar.activation(
                            Ehat[:], c_cm, AF.Exp,
                            bias=c_cm[:, C - 1:C], scale=-1.0,
                        )
                        Khat = rc.tile([128, C], b16, tag="Khat")
                        nc.vector.tensor_mul(Khat[:], k_cm[:], Ehat[:])
                        khp = rcps_b.tile([128, C], b16, tag="psb")
                        nc.tensor.transpose(khp[:], Khat[:], idb_sb[:])
                        Khat_tm = rc.tile([128, C], b16, tag="Khat_tm")
                        nc.vector.tensor_copy(Khat_tm[:], khp[:])
                        lamc = rc.tile([128, 1], f32, tag="lamc")
                        nc.scalar.activation(lamc[:], c_cm[:, C - 1:C], AF.Exp)

                        # R = v - Ktil @ S
                        mem_ps = rcps_f.tile([128, C], f32, tag="psf")
                        nc.tensor.matmul(
                            mem_ps[:], Ktil[:], Sb[s][:], start=True, stop=True
                        )
                        Z = rc.tile([128, C], b16, tag="Zsolve")
                        nc.vector.tensor_tensor(Z[:], v_tm[:], mem_ps[:], sub)

                        # Neumann doubling: Z <- Z + Apow Z ; Apow <- Apow^2
                        Apow, ApowT = A_sb, At
                        for lev in range(KLEV + 1):
                            zp = rcps_f.tile([128, C], f32, tag="psf")
                            nc.scalar.copy(zp[:], Z[:])
                            nc.tensor.matmul(
                                zp[:], ApowT[:], Z[:], start=False, stop=True,
                                skip_group_check=True,
                            )
                            Z = rc.tile([128, C], b16, tag="Zsolve")
                            nc.vector.tensor_copy(Z[:], zp[:])
                            if lev < KLEV:
                                a2p = rcps_f.tile([128, C], f32, tag="psf")
                                nc.tensor.matmul(
                                    a2p[:], ApowT[:], Apow[:], start=True, stop=True
                                )
                                Apow = rc.tile([128, C], b16, tag="A_sb")
                                nc.vector.tensor_copy(Apow[:], a2p[:])
                                a2tp = rcps_b.tile([128, C], b16, tag="psb")
                                nc.tensor.transpose(a2tp[:], Apow[:], idb_sb[:])
                                ApowT = rc.tile([128, C], b16, tag="At")
                                nc.vector.tensor_copy(ApowT[:], a2tp[:])
                        U = rc.tile([128, C], b16, tag="U")
                        nc.vector.tensor_scalar_mul(U[:], Z[:], bcols[:, 0:1])

                        # o = Qtil^T S + P U
                        o_ps = rcps_f.tile([128, C], f32, tag="psf")
                        nc.tensor.matmul(o_ps[:], Qtil[:], Sb[s][:], start=True, stop=False)
                        nc.tensor.matmul(
                            o_ps[:], Pt[:], U[:], start=False, stop=True,
                            skip_group_check=True,
                        )
                        o_bf = rc.tile([128, C], b16, tag="o_bf")
                        nc.vector.tensor_copy(o_bf[:], o_ps[:])
                        nc.sync.dma_start(ot[csl, rsl], o_bf[:])

                        # S update: S = lamc*S + Khat^T U
                        sps = rcps_f.tile([128, C], f32, tag="psf")
                        nc.vector.tensor_scalar_mul(sps[:], Sf[s][:], lamc[:])
                        nc.tensor.matmul(
                            sps[:], Khat_tm[:], U[:], start=False, stop=True,
                            skip_group_check=True,
                        )
                        nc.vector.tensor_copy(Sf[s][:], sps[:])
                        nc.vector.tensor_copy(Sb[s][:], sps[:])

            # ============ Phase C2: RMS norm + sigmoid gate ============
            with (
                tc.tile_pool(name="c2", bufs=3) as c2p,
                tc.tile_pool(name="c2ps", bufs=3, space="PSUM") as c2ps,
            ):
                for r in range(32):
                    tsl = slice(r * 128, (r + 1) * 128)
                    o2 = c2p.tile([128, 256], b16, tag="o2")
                    nc.sync.dma_start(o2[:], ot[tsl, :])
                    for h in range(2):
                        hsl2 = slice(h * 128, (h + 1) * 128)
                        osq = c2p.tile([128, 128], b16, tag="osq")
                        ss = c2p.tile([128, 1], f32, tag="ss")
                        nc.scalar.activation(
                            osq[:], o2[:, hsl2], AF.Square, accum_out=ss[:]
                        )
                        rt2 = c2p.tile([128, 1], f32, tag="rt2")
                        nc.scalar.activation(
                            rt2[:], ss[:], AF.Sqrt, bias=epst[:, 0:1], scale=1.0 / D
                        )
                        rr2 = c2p.tile([128, 1], f32, tag="rr2")
                        nc.vector.reciprocal(rr2[:], rt2[:])
                        o_n = c2p.tile([128, 128], b16, tag="o_n")
                        nc.vector.tensor_scalar_mul(o_n[:], o2[:, hsl2], rr2[:])
                        onp = c2ps.tile([128, 128], b16, tag="onp")
                        nc.tensor.transpose(onp[:], o_n[:], idb_sb[:])
                        sgt = c2p.tile([128, 128], b16, tag="sgt")
                        nc.sync.dma_start(sgt[:], sg[hsl2, tsl])
                        ogt2 = c2p.tile([128, 128], b16, tag="ogt2")
                        nc.vector.tensor_mul(ogt2[:], onp[:], sgt[:])
                        nc.sync.dma_start(og[hsl2, tsl], ogt2[:])

            # ================= Phase D: output projection =================
            with (
                tc.tile_pool(name="op", bufs=4) as opool,
                tc.tile_pool(name="opps", bufs=4, space="PSUM") as oppool,
                tc.tile_pool(name="wop", bufs=1) as wopool,
            ):
                wo_t = []
                for c2 in range(2):
                    wot = wopool.tile([128, HID], b16, tag=f"wo{c2}")
                    nc.sync.dma_start(wot[:], wo[c2 * 128:(c2 + 1) * 128, :])
                    wo_t.append(wot)
                for tt in range(32):
                    ogt = []
                    for c2 in range(2):
                        t = opool.tile([128, 128], b16, tag="ogtile")
                        nc.sync.dma_start(
                            t[:], og[c2 * 128:(c2 + 1) * 128, tt * 128:(tt + 1) * 128]
                        )
                        ogt.append(t)
                    outsb = opool.tile([128, HID], b16, tag="outsb")
                    for n4 in range(4):
                        ps = oppool.tile([128, 512], f32, tag="ops")
                        for c2 in range(2):
                            nc.tensor.matmul(
                                ps[:], ogt[c2][:],
                                wo_t[c2][:, n4 * 512:(n4 + 1) * 512],
                                start=(c2 == 0), stop=(c2 == 1),
                            )
                        nc.scalar.copy(outsb[:, n4 * 512:(n4 + 1) * 512], ps[:])
                    nc.sync.dma_start(opf[tt * 128:(tt + 1) * 128, :], outsb[:])
                nc.gpsimd.collective_compute(
                    "ReduceScatter", mybir.AluOpType.add,
                    replica_groups=[list(range(8))],
                    ins=[opf[:]], outs=[rso[:]],
                )
                nc.sync.dma_start(out[:], rso[:])

    nc.compile()
    return nc


def _chmaj(w):  # [256, 4] -> [128, 8] (ch-chunk along cols)
    return np.ascontiguousarray(w.reshape(2, 128, 4).transpose(1, 0, 2).reshape(128, 8)).astype(np.float32)


def _prep_inputs(inputs):
    f32 = np.float32
    hs = np.asarray(inputs["hidden_states"], f32).reshape(N, HID)
    hst = np.ascontiguousarray(hs.T).astype(bf16)       # [HID, N]

    def tile_w(w_t):  # [HID, M] -> [128, 16*M] kt-tiled
        m = w_t.shape[1]
        return np.ascontiguousarray(
            w_t.reshape(16, 128, m).transpose(1, 0, 2).reshape(128, 16 * m)
        ).astype(bf16)

    Wq, Wk, Wv = (np.asarray(inputs[x], f32) for x in ("Wq", "Wk", "Wv"))
    Wfa, Wfb = np.asarray(inputs["Wfa"], f32), np.asarray(inputs["Wfb"], f32)
    Wga, Wgb = np.asarray(inputs["Wga"], f32), np.asarray(inputs["Wgb"], f32)
    Wo = np.asarray(inputs["Wo"], f32)
    onw = np.asarray(inputs["o_norm_w"], f32)
    Wo_fold = Wo * np.tile(onw, H)[None, :]
    A = np.asarray(inputs["A_log"], f32).reshape(H)
    dt_bias = np.asarray(inputs["dt_bias"], f32)
    beta_all = 1.0 / (1.0 + np.exp(-(hs @ np.asarray(inputs["Wb"], f32).T)))  # [N, H]
    gpre = (hs @ Wfa.T) @ Wfb.T + dt_bias[None, :]
    aneg_full = np.repeat(-np.exp(A), D)[None, :]
    g_full = (aneg_full * np.logaddexp(0.0, gpre)).astype(f32)     # [N, P]

    Lx = np.triu(np.ones((128, 128), f32))              # L[j,t] = 1 if j <= t
    mS = np.tril(np.ones((128, 128), f32), -1).astype(bf16)
    mI = np.tril(np.ones((128, 128), f32), 0).astype(bf16)
    idb = np.eye(128, dtype=f32).astype(bf16)
    idf = np.eye(128, dtype=f32)
    pm1 = np.array([[1.0, -1.0]], f32)

    in_maps = []
    for core in range(8):
        cs = slice(core * 256, (core + 1) * 256)
        hsl = slice(core * 2, core * 2 + 2)
        im = {
            "hst": hst,
            "wq": tile_w(np.ascontiguousarray(Wq[cs].T)),
            "wk": tile_w(np.ascontiguousarray(Wk[cs].T)),
            "wv": tile_w(np.ascontiguousarray(Wv[cs].T)),
            "wga": tile_w(np.ascontiguousarray(Wga.T)),
            "wgb": np.ascontiguousarray(Wgb[cs].T).astype(bf16),
            "gs": np.ascontiguousarray(g_full[:, cs].T),
            "wo": np.ascontiguousarray(Wo_fold[:, cs].T).astype(bf16),
            "cwq": _chmaj(np.asarray(inputs["wq_conv"], f32)[cs]),
            "cwk": _chmaj(np.asarray(inputs["wk_conv"], f32)[cs]),
            "cwv": _chmaj(np.asarray(inputs["wv_conv"], f32)[cs]),
            "betah": np.ascontiguousarray(beta_all[:, hsl].T).astype(f32),
            "lx": Lx, "maskS": mS, "maskI": mI, "idb": idb, "idf": idf, "pm1": pm1,
        }
        in_maps.append(im)
    return in_maps


def _get_nc():
    if "nc" not in _CACHED:
        _CACHED["nc"] = _build_nc()
    return _CACHED["nc"]


def kernel(**inputs):
    _config_jax_cache()
    from concourse.bass_utils import run_bass_kernel_spmd

    nc = _get_nc()
    in_maps = _prep_inputs(inputs)
    res = run_bass_kernel_spmd(nc, in_maps, list(range(8)))
    acc = np.concatenate(
        [np.asarray(r["out"], dtype=np.float32) for r in res.results], axis=0
    )
    return acc.reshape(B, T, HID)


# revision 30
# speedup vs baseline: 19.9005x; 1.1469x over previous
"""KimiDeltaAttention — Trainium2 Bass kernel, 8-core head-sharded SPMD.

Each core handles 2 of 16 heads (256 of 2048 channels):
  - q/k/v projections + causal depthwise conv + silu  (bf16 matmuls, fp32 psum)
  - decay gate g = -exp(A_log) * softplus(low-rank proj + dt_bias)   (fp32)
  - l2norm(q)*D^-0.5, l2norm(k)
  - chunked gated-delta-rule recurrence (chunk C=128, sub-chunk SC=4,
    block-start boundary factorization so every exp arg is <= +66; W/P via
    32 row-block matmuls; (I + W diag(beta))^{-1} via Neumann doubling)
  - gated RMSNorm (sigmoid low-rank gate), output projection partial
Host: shards inputs, computes beta (tiny), sums the 8 partial outputs.

Shapes hardcoded: B=2, T=2048, HID=2048, H=16, D=128, K=4.
"""

import os

os.environ.setdefault("JAX_COMPILATION_CACHE_DIR", "/root/jax_cache")
os.environ.setdefault("JAX_PERSISTENT_CACHE_MIN_ENTRY_SIZE_BYTES", "-1")
os.environ.setdefault("JAX_PERSISTENT_CACHE_MIN_COMPILE_TIME_SECS", "0")

import sys

if "/opt/trn_rl_repo" not in sys.path:
    sys.path.insert(0, "/opt/trn_rl_repo")

import numpy as np
import ml_dtypes

bf16 = ml_dtypes.bfloat16

B, T, HID = 2, 2048, 2048
H, D = 16, 128
P = H * D
N = B * T            # 4096 tokens
C = 128              # chunk length
SC = 4               # row-block
WIN = 64             # col window per row-block
NBLK = C // SC       # 32
NCH = T // C         # 16 chunks per sequence
EPS = 1e-6
KLEV = 4             # Neumann doubling levels (A^1..A^16)
BIGNEG_PAD = 3.0e38  # left-pad value so exp(a - pad) == 0

_CACHED = {}


def _config_jax_cache():
    try:
        import jax
        jax.config.update("jax_compilation_cache_dir", "/root/jax_cache")
        jax.config.update("jax_persistent_cache_min_entry_size_bytes", -1)
        jax.config.update("jax_persistent_cache_min_compile_time_secs", 0)
    except Exception:
        pass


def _build_nc():
    import concourse.bass as bass
    import concourse.tile as tile
    from concourse import bacc, mybir

    f32 = mybir.dt.float32
    b16 = mybir.dt.bfloat16

    nc = bacc.Bacc("TRN2", target_bir_lowering=False, debug=False, num_devices=8)

    # ---- I/O ----
    hsts = nc.declare_dram_parameter("hsts", [HID, N // 8], b16, isOutput=False)   # hs^T token-shard
    wq = nc.declare_dram_parameter("wq", [128, 16 * 256], b16, isOutput=False)     # kt-tiled W^T
    wk = nc.declare_dram_parameter("wk", [128, 16 * 256], b16, isOutput=False)
    wv = nc.declare_dram_parameter("wv", [128, 16 * 256], b16, isOutput=False)
    wga = nc.declare_dram_parameter("wga", [128, 16 * 128], b16, isOutput=False)
    wgb = nc.declare_dram_parameter("wgb", [128, 256], b16, isOutput=False)
    gs = nc.declare_dram_parameter("gs", [256, N], f32, isOutput=False)
    wo = nc.declare_dram_parameter("wo", [256, HID], b16, isOutput=False)          # rows=channels
    cwq = nc.declare_dram_parameter("cwq", [128, 8], f32, isOutput=False)
    cwk = nc.declare_dram_parameter("cwk", [128, 8], f32, isOutput=False)
    cwv = nc.declare_dram_parameter("cwv", [128, 8], f32, isOutput=False)
    betah = nc.declare_dram_parameter("betah", [2, N], f32, isOutput=False)
    lx = nc.declare_dram_parameter("lx", [128, 128], f32, isOutput=False)          # L[j,t]=1 if j<=t
    maskS = nc.declare_dram_parameter("maskS", [128, 128], b16, isOutput=False)    # strict lower
    maskI = nc.declare_dram_parameter("maskI", [128, 128], b16, isOutput=False)    # incl lower
    idb = nc.declare_dram_parameter("idb", [128, 128], b16, isOutput=False)        # identity bf16
    idf = nc.declare_dram_parameter("idf", [128, 128], f32, isOutput=False)        # identity f32
    pm1 = nc.declare_dram_parameter("pm1", [1, 2], f32, isOutput=False)            # [+1, -1]
    out = nc.declare_dram_parameter("out", [N // 8, HID], b16, isOutput=True)

    # DRAM scratch
    qs = nc.dram_tensor("qs", [256, N], b16)
    ks = nc.dram_tensor("ks", [256, N], b16)
    vs = nc.dram_tensor("vs", [256, N], b16)
    ot = nc.dram_tensor("ot", [N, 256], b16)
    hsb = nc.dram_tensor("hsb", [HID, N // 8], b16)
    hsg = nc.dram_tensor("hsg", [8 * HID, N // 8], b16, addr_space="Shared")
    hst = nc.dram_tensor("hst", [HID, N], b16)
    opf = nc.dram_tensor("opf", [N, HID], b16)
    rso = nc.dram_tensor("rso", [N // 8, HID], b16)
    wdn = nc.dram_tensor("wdn", [4, 128, 192], b16)
    pdn = nc.dram_tensor("pdn", [4, 128, 192], b16)
    sg = nc.dram_tensor("sg", [256, N], b16)
    og = nc.dram_tensor("og", [256, N], b16)

    import bass_rust

    def pat(ap, offset_elems, dims, dtype_bytes):
        """Custom free-dim pattern on a 2D sbuf tile ap (keeps partition dim)."""
        c = ap.copy()
        part = list(c.ap)[0]
        c.ap = bass_rust.VecI64Pair([list(part)] + [list(d) for d in dims])
        c.offset = ap.offset + offset_elems
        return c

    with tile.TileContext(nc) as tc:
        with tc.tile_pool(name="consts", bufs=1) as cpool:
            lx_sb = cpool.tile([128, 128], f32)
            nc.sync.dma_start(lx_sb[:], lx[:])
            mS_sb = cpool.tile([128, 128], b16)
            nc.sync.dma_start(mS_sb[:], maskS[:])
            mI_sb = cpool.tile([128, 128], b16)
            nc.sync.dma_start(mI_sb[:], maskI[:])
            idb_sb = cpool.tile([128, 128], b16)
            nc.sync.dma_start(idb_sb[:], idb[:])
            idf_sb = cpool.tile([128, 128], f32)
            nc.sync.dma_start(idf_sb[:], idf[:])
            pm1_sb = cpool.tile([1, 2], f32)
            nc.sync.dma_start(pm1_sb[:], pm1[:])
            beta0 = cpool.tile([1, N], f32)
            nc.sync.dma_start(beta0[:], betah[0:1, :])
            beta1 = cpool.tile([1, N], f32)
            nc.sync.dma_start(beta1[:], betah[1:2, :])
            beta_t = (beta0, beta1)
            zro = cpool.tile([128, 192], b16)
            nc.gpsimd.memset(zro[:], 0.0)
            epst = cpool.tile([128, 2], f32)
            nc.gpsimd.memset(epst[:, 0:1], EPS)
            nc.gpsimd.memset(epst[:, 1:2], EPS * D)
            cw_sb = {}
            for nm, t in (("q", cwq), ("k", cwk), ("v", cwv)):
                cwt = cpool.tile([128, 8], f32, tag=f"cw{nm}")
                cw_sb[nm] = cwt
                nc.sync.dma_start(cw_sb[nm][:], t[:])

            # ---- AllGather hs^T token shards -> full hst ----
            nc.sync.dma_start(hsb[:], hsts[:])
            nc.gpsimd.collective_compute(
                "AllGather", mybir.AluOpType.bypass,
                replica_groups=[list(range(8))],
                ins=[hsb[:]], outs=[hsg[:]],
            )
            for sh in range(8):
                nc.sync.dma_start(
                    hst[:, sh * 512:(sh + 1) * 512],
                    hsg[sh * HID:(sh + 1) * HID, :],
                )

            # ================= Phase B: projections =================
            SEG = 2048 + 3
            with (
                tc.tile_pool(name="hts", bufs=1) as hpool,
                tc.tile_pool(name="wts", bufs=1) as wpool,
                tc.tile_pool(name="xp", bufs=1) as xpool,
                tc.tile_pool(name="proj", bufs=1) as ppool,
                tc.tile_pool(name="nrm", bufs=1) as npool,
                tc.tile_pool(name="projps", bufs=2, space="PSUM") as pps,
            ):
                w_sb = {}
                for nm, t in (("q", wq), ("k", wk), ("v", wv)):
                    wt_ = wpool.tile([128, 16 * 256], b16, tag=f"w{nm}")
                    w_sb[nm] = wt_
                    nc.sync.dma_start(w_sb[nm][:], t[:])
                for nm, t in (("ga", wga),):
                    wt_ = wpool.tile([128, 16 * 128], b16, tag=f"w{nm}")
                    w_sb[nm] = wt_
                    nc.sync.dma_start(w_sb[nm][:], t[:])
                for nm, t in (("gb", wgb),):
                    wt_ = wpool.tile([128, 256], b16, tag=f"w{nm}")
                    w_sb[nm] = wt_
                    nc.sync.dma_start(w_sb[nm][:], t[:])
                ones_col = wpool.tile([128, 1], b16, tag="ones")
                nc.gpsimd.memset(ones_col[:], 1.0)
                ones_row = wpool.tile([1, 128], b16, tag="onesr")
                nc.gpsimd.memset(ones_row[:], 1.0)

                xpads = {}
                for nm in ("q", "k", "v"):
                    for ch in range(2):
                        xp = xpool.tile([128, 2 * SEG], b16, tag=f"xp{nm}{ch}")
                        nc.gpsimd.memset(xp[:, 0:3], 0.0)
                        nc.gpsimd.memset(xp[:, SEG:SEG + 3], 0.0)
                        xpads[(nm, ch)] = xp
                lowr = {}
                for nm in ("ga",):
                    lrt = xpool.tile([128, N], b16, tag=f"lr{nm}")
                    lowr[nm] = lrt

                for nt in range(8):
                    hts = []
                    for kt in range(16):
                        ht = hpool.tile([128, 512], b16, tag=f"ht{kt}")
                        nc.sync.dma_start(
                            ht[:], hst[kt * 128:(kt + 1) * 128, nt * 512:(nt + 1) * 512]
                        )
                        hts.append(ht)
                    for nm in ("q", "k", "v"):
                        for ch in range(2):
                            ps = pps.tile([128, 512], f32, tag="pp")
                            for kt in range(16):
                                nc.tensor.matmul(
                                    ps[:],
                                    w_sb[nm][:, kt * 256 + ch * 128: kt * 256 + (ch + 1) * 128],
                                    hts[kt][:],
                                    start=(kt == 0), stop=(kt == 15),
                                )
                            b = nt // 4
                            col = b * SEG + 3 + (nt % 4) * 512
                            nc.scalar.copy(xpads[(nm, ch)][:, col:col + 512], ps[:])
                    for nm in ("ga",):
                        ps = pps.tile([128, 512], f32, tag="pp")
                        for kt in range(16):
                            nc.tensor.matmul(
                                ps[:], w_sb[nm][:, kt * 128:(kt + 1) * 128],
                                hts[kt][:], start=(kt == 0), stop=(kt == 15),
                            )
                        nc.scalar.copy(lowr[nm][:, nt * 512:(nt + 1) * 512], ps[:])

                # ---- gate: sg (bf16) ----
                for ch in range(2):
                    for nt in range(8):
                        ps2 = pps.tile([128, 512], f32, tag="pp")
                        nc.tensor.matmul(
                            ps2[:], w_sb["gb"][:, ch * 128:(ch + 1) * 128],
                            lowr["ga"][:, nt * 512:(nt + 1) * 512],
                            start=True, stop=True,
                        )
                        sgb = ppool.tile([128, 512], b16, tag="sgb")
                        nc.scalar.activation(
                            sgb[:], ps2[:], mybir.ActivationFunctionType.Sigmoid
                        )
                        nc.sync.dma_start(
                            sg[ch * 128:(ch + 1) * 128, nt * 512:(nt + 1) * 512], sgb[:]
                        )

                # ---- conv + silu (+ l2norm for q,k) ----
                mult = mybir.AluOpType.mult
                add = mybir.AluOpType.add
                for ch in range(2):
                    for nm, dst in (("q", qs), ("k", ks), ("v", vs)):
                        xpad = xpads[(nm, ch)]
                        wcol = cw_sb[nm][:, ch * 4:(ch + 1) * 4]
                        y = ppool.tile([128, N], b16, tag="ysb")
                        for b in range(2):
                            ysl = y[:, b * 2048:(b + 1) * 2048]
                            xb = xpad[:, b * SEG: (b + 1) * SEG]
                            nc.vector.tensor_scalar_mul(ysl, xb[:, 3:2051], wcol[:, 3:4])
                            for tau in (2, 1, 0):
                                nc.vector.scalar_tensor_tensor(
                                    ysl, xb[:, tau:tau + 2048], wcol[:, tau:tau + 1],
                                    ysl, mult, add,
                                )
                        ysil = ppool.tile([128, N], b16, tag="ysil")
                        nc.scalar.activation(
                            ysil[:], y[:], mybir.ActivationFunctionType.Silu
                        )
                        if nm == "v":
                            nc.sync.dma_start(dst[ch * 128:(ch + 1) * 128, :], ysil[:])
                            continue
                        # l2norm over d (partitions) via ones-matmul
                        ysq = ppool.tile([128, N], b16, tag="ysb")
                        nc.scalar.activation(
                            ysq[:], ysil[:], mybir.ActivationFunctionType.Square
                        )
                        ssq = npool.tile([1, N], f32, tag="ssq")
                        for nt in range(8):
                            ssp = pps.tile([1, 512], f32, tag="ssp")
                            nc.tensor.matmul(
                                ssp[:], ones_col[:],
                                ysq[:, nt * 512:(nt + 1) * 512],
                                start=True, stop=True,
                            )
                            nc.scalar.copy(ssq[:, nt * 512:(nt + 1) * 512], ssp[:])
                        scl = float(D) if nm == "q" else 1.0
                        bcol = epst[0:1, 1:2] if nm == "q" else epst[0:1, 0:1]
                        nc.scalar.activation(
                            ssq[:], ssq[:], mybir.ActivationFunctionType.Sqrt,
                            bias=bcol, scale=scl,
                        )
                        rr = npool.tile([1, N], b16, tag="rr")
                        with nc.allow_low_precision(reason="bf16 norm scale"):
                            nc.vector.reciprocal(rr[:], ssq[:])
                        yn = ppool.tile([128, N], b16, tag="ysb")
                        for nt in range(8):
                            sl = slice(nt * 512, (nt + 1) * 512)
                            bb = pps.tile([128, 512], f32, tag="pp")
                            nc.tensor.matmul(
                                bb[:], ones_row[:], rr[:, sl], start=True, stop=True
                            )
                            nc.vector.tensor_mul(yn[:, sl], ysil[:, sl], bb[:])
                        nc.sync.dma_start(dst[ch * 128:(ch + 1) * 128, :], yn[:])

            # ================= Phase C: recurrence =================
            mult = mybir.AluOpType.mult
            add = mybir.AluOpType.add
            sub = mybir.AluOpType.subtract
            AF = mybir.ActivationFunctionType
            with (
                tc.tile_pool(name="state", bufs=1) as spool,
                tc.tile_pool(name="rc", bufs=3) as rc,
                tc.tile_pool(name="rcpsf", bufs=3, space="PSUM") as rcps_f,
                tc.tile_pool(name="rcpsb", bufs=2, space="PSUM") as rcps_b,
                tc.tile_pool(name="wpps", bufs=2, space="PSUM") as wpps,
            ):
                Sf = []
                Sb = []
                for s in range(4):
                    sf = spool.tile([128, 128], f32, tag=f"Sf{s}")
                    nc.gpsimd.memset(sf[:], 0.0)
                    Sf.append(sf)
                    sbt = spool.tile([128, 128], b16, tag=f"Sb{s}")
                    nc.gpsimd.memset(sbt[:], 0.0)
                    Sb.append(sbt)

                for n in range(NCH):
                    for s in range(4):
                        b, h = s // 2, s % 2
                        t0 = b * T + n * C
                        rsl = slice(h * 128, (h + 1) * 128)
                        csl = slice(t0, t0 + C)

                        q_cm = rc.tile([128, C], b16, tag="q_cm")
                        nc.sync.dma_start(q_cm[:], qs[rsl, csl])
                        k_cm = rc.tile([128, C], b16, tag="k_cm")
                        nc.sync.dma_start(k_cm[:], ks[rsl, csl])
                        v_tm = rc.tile([128, C], b16, tag="v_tm")
                        nc.sync.dma_start_transpose(v_tm[:], vs[rsl, csl])
                        g_cm = rc.tile([128, C], f32, tag="g_cm")
                        nc.sync.dma_start(g_cm[:], gs[rsl, csl])
                        gtp = rcps_f.tile([128, C], f32, tag="psf")
                        nc.tensor.transpose(gtp[:], g_cm[:], idf_sb[:])
                        g_tm = rc.tile([128, C], f32, tag="g_tm")
                        nc.vector.tensor_copy(g_tm[:], gtp[:])

                        # beta columns: [128, 2] = (beta, -beta)
                        bps = rcps_f.tile([128, 2], f32, tag="psf")
                        nc.tensor.matmul(
                            bps[:], beta_t[h][:, csl], pm1_sb[:],
                            start=True, stop=True,
                        )
                        bcols = rc.tile([128, 2], f32, tag="bcols")
                        nc.scalar.copy(bcols[:], bps[:])

                        # cumsum c = L @ g_tm  -> c_tm -> transpose -> cpad
                        cps = rcps_f.tile([128, C], f32, tag="psf")
                        nc.tensor.matmul(cps[:], lx_sb[:], g_tm[:], start=True, stop=True)
                        c_tm = rc.tile([128, C], f32, tag="c_tm")
                        nc.scalar.copy(c_tm[:], cps[:])
                        ctp = rcps_f.tile([128, C], f32, tag="psf")
                        nc.tensor.transpose(ctp[:], c_tm[:], idf_sb[:])
                        cpad = rc.tile([128, 61 + C], f32, tag="cpad")
                        nc.gpsimd.memset(cpad[:, 0:60], BIGNEG_PAD)
                        nc.gpsimd.memset(cpad[:, 60:61], 0.0)
                        nc.vector.tensor_copy(cpad[:, 61:61 + C], ctp[:])
                        c_cm = cpad[:, 61:61 + C]

                        kqpad = rc.tile([128, 60 + C], b16, tag="kqpad")
                        nc.gpsimd.memset(kqpad[:, 0:60], 0.0)
                        nc.vector.tensor_copy(kqpad[:, 60:60 + C], k_cm[:])

                        # colD[d,(I,jj)] = c_j - a_I  (j = 4I-60+jj), 32 TS ops
                        colD = rc.tile([128, NBLK * WIN], f32, tag="colD")
                        for I in range(NBLK):
                            nc.vector.tensor_scalar_sub(
                                colD[:, I * WIN:(I + 1) * WIN],
                                cpad[:, 4 * I + 1:4 * I + 65],
                                cpad[:, 60 + 4 * I:61 + 4 * I],
                            )
                        colE = rc.tile([128, NBLK * WIN], b16, tag="colE")
                        nc.scalar.activation(colE[:], colD[:], AF.Exp, scale=-1.0)
                        k_w = pat(kqpad[:], 0, [[SC, NBLK], [1, WIN]], 2)
                        colK = rc.tile([128, NBLK * WIN], b16, tag="colK")
                        nc.vector.tensor_tensor(colK[:], colE[:], k_w, mult)

                        # rowE = 1/colE on in-block cols (e^{c-a} = 1/e^{a-c})
                        rowE = rc.tile([128, C], b16, tag="rowE")
                        inblk = pat(colE[:], WIN - SC, [[WIN, NBLK], [1, SC]], 2)
                        with nc.allow_low_precision(reason="bf16 rowE"):
                            nc.vector.reciprocal(rowE[:], inblk)
                        rowKQ = rc.tile([128, NBLK * 2 * SC], b16, tag="rowKQ")
                        dst_k = pat(rowKQ[:], 0, [[2 * SC, NBLK], [1, SC]], 2)
                        dst_q = pat(rowKQ[:], SC, [[2 * SC, NBLK], [1, SC]], 2)
                        src4 = lambda ap: pat(ap, 0, [[SC, NBLK], [1, SC]], 2)
                        nc.vector.tensor_tensor(dst_k, k_cm[:], src4(rowE[:]), mult)
                        nc.vector.tensor_tensor(dst_q, q_cm[:], src4(rowE[:]), mult)

                        # W/P row-block matmuls, 4 groups of 8 blocks
                        whalo = rc.tile([128, 192], b16, tag="whalo")
                        phalo = rc.tile([128, 192], b16, tag="phalo")
                        nc.gpsimd.memset(whalo[:], 0.0)
                        nc.gpsimd.memset(phalo[:], 0.0)
                        stage = rc.tile([8, 2048], b16, tag="wpstage")
                        for grp in range(4):
                            wp_ps = wpps.tile([128, 512], f32, tag="wp_ps")
                            for Ii in range(8):
                                I = grp * 8 + Ii
                                nc.tensor.matmul(
                                    wp_ps[0:2 * SC, Ii * WIN:(Ii + 1) * WIN],
                                    rowKQ[:, I * 2 * SC:(I + 1) * 2 * SC],
                                    colK[:, I * WIN:(I + 1) * WIN],
                                    start=True, stop=True,
                                )
                            nc.scalar.copy(
                                stage[0:8, grp * 512:(grp + 1) * 512], wp_ps[0:8, :]
                            )
                        # banded scatter via DRAM (flat addressing)
                        psg = list(stage[:].ap)[0][0]
                        nc.sync.dma_start(wdn[s], zro[:])
                        nc.sync.dma_start(pdn[s], zro[:])
                        for mat, dnt in ((0, wdn), (1, pdn)):
                            dst = dnt[s].copy()
                            dst.ap = bass_rust.VecI64Pair(
                                [[192, 4], [4 * 192 + 4, 32], [1, 64]])
                            srcb = pat(stage[:], mat * 4 * psg, [], 2)
                            srcb.ap = bass_rust.VecI64Pair(
                                [[psg, 4], [64, 32], [1, 64]])
                            nc.sync.dma_start(dst, srcb)
                        nc.sync.dma_start(whalo[:], wdn[s])
                        nc.sync.dma_start(phalo[:], pdn[s])
                        W_sb = rc.tile([128, C], b16, tag="W_sb")
                        P_sb = rc.tile([128, C], b16, tag="P_sb")
                        nc.vector.tensor_mul(W_sb[:], whalo[:, 60:188], mS_sb[:])
                        nc.vector.tensor_mul(P_sb[:], phalo[:, 60:188], mI_sb[:])

                        # A^T = -diag(beta) W^T ; A = transpose(A^T)
                        wtp = rcps_b.tile([128, C], b16, tag="psb")
                        nc.tensor.transpose(wtp[:], W_sb[:], idb_sb[:])
                        At = rc.tile([128, C], b16, tag="At")
                        nc.vector.tensor_scalar_mul(At[:], wtp[:], bcols[:, 1:2])
                        atp = rcps_b.tile([128, C], b16, tag="psb")
                        nc.tensor.transpose(atp[:], At[:], idb_sb[:])
                        A_sb = rc.tile([128, C], b16, tag="A_sb")
                        nc.vector.tensor_copy(A_sb[:], atp[:])
                        ptp = rcps_b.tile([128, C], b16, tag="psb")
                        nc.tensor.transpose(ptp[:], P_sb[:], idb_sb[:])
                        Pt = rc.tile([128, C], b16, tag="Pt")
                        nc.vector.tensor_copy(Pt[:], ptp[:])

                        # E128 = exp(c); Ktil/Qtil; Ehat/Khat; LamC
                        E128 = rc.tile([128, C], b16, tag="E128")
                        nc.scalar.activation(E128[:], c_cm, AF.Exp)
                        Ktil = rc.tile([128, C], b16, tag="Ktil")
                        nc.vector.tensor_mul(Ktil[:], k_cm[:], E128[:])
                        Qtil = rc.tile([128, C], b16, tag="Qtil")
                        nc.vector.tensor_mul(Qtil[:], q_cm[:], E128[:])
                        Ehat = rc.tile([128, C], b16, tag="Ehat")
                        nc.scalar.activation(
                            Ehat[:], c_cm, AF.Exp,
                            bias=c_cm[:, C - 1:C], scale=-1.0,
                        )
                        Khat = rc.tile([128, C], b16, tag="Khat")
                        nc.vector.tensor_mul(Khat[:], k_cm[:], Ehat[:])
                        khp = rcps_b.tile([128, C], b16, tag="psb")
                        nc.tensor.transpose(khp[:], Khat[:], idb_sb[:])
                        Khat_tm = rc.tile([128, C], b16, tag="Khat_tm")
                        nc.vector.tensor_copy(Khat_tm[:], khp[:])
                        lamc = rc.tile([128, 1], f32, tag="lamc")
                        nc.scalar.activation(lamc[:], c_cm[:, C - 1:C], AF.Exp)

                        # R = v - Ktil @ S
                        mem_ps = rcps_f.tile([128, C], f32, tag="psf")
                        nc.tensor.matmul(
                            mem_ps[:], Ktil[:], Sb[s][:], start=True, stop=True
                        )
                        Z = rc.tile([128, C], b16, tag="Zsolve")
                        nc.vector.tensor_tensor(Z[:], v_tm[:], mem_ps[:], sub)

                        # Neumann doubling: Z <- Z + Apow Z ; Apow <- Apow^2
                        Apow, ApowT = A_sb, At
                        for lev in range(KLEV + 1):
                            zp = rcps_f.tile([128, C], f32, tag="psf")
                            nc.scalar.copy(zp[:], Z[:])
                            nc.tensor.matmul(
                                zp[:], ApowT[:], Z[:], start=False, stop=True,
                                skip_group_check=True,
                            )
                            Z = rc.tile([128, C], b16, tag="Zsolve")
                            nc.vector.tensor_copy(Z[:], zp[:])
                            if lev < KLEV:
                                a2p = rcps_f.tile([128, C], f32, tag="psf")
                                nc.tensor.matmul(
                                    a2p[:], ApowT[:], Apow[:], start=True, stop=True
                                )
                                Apow = rc.tile([128, C], b16, tag="A_sb")
                                nc.vector.tensor_copy(Apow[:], a2p[:])
                                a2tp = rcps_b.tile([128, C], b16, tag="psb")
                                nc.tensor.transpose(a2tp[:], Apow[:], idb_sb[:])
                                ApowT = rc.tile([128, C], b16, tag="At")
                                nc.vector.tensor_copy(ApowT[:], a2tp[:])
                        U = rc.tile([128, C], b16, tag="U")
                        nc.vector.tensor_scalar_mul(U[:], Z[:], bcols[:, 0:1])

                        # o = Qtil^T S + P U
                        o_ps = rcps_f.tile([128, C], f32, tag="psf")
                        nc.tensor.matmul(o_ps[:], Qtil[:], Sb[s][:], start=True, stop=False)
                        nc.tensor.matmul(
                            o_ps[:], Pt[:], U[:], start=False, stop=True,
                            skip_group_check=True,
                        )
                        o_bf = rc.tile([128, C], b16, tag="o_bf")
                        nc.vector.tensor_copy(o_bf[:], o_ps[:])
                        nc.sync.dma_start(ot[csl, rsl], o_bf[:])

                        # S update: S = lamc*S + Khat^T U
                        sps = rcps_f.tile([128, C], f32, tag="psf")
                        nc.vector.tensor_scalar_mul(sps[:], Sf[s][:], lamc[:])
                        nc.tensor.matmul(
                            sps[:], Khat_tm[:], U[:], start=False, stop=True,
                            skip_group_check=True,
                        )
                        nc.vector.tensor_copy(Sf[s][:], sps[:])
                        nc.vector.tensor_copy(Sb[s][:], sps[:])

            # ============ Phase C2: RMS norm + sigmoid gate ============
            with (
                tc.tile_pool(name="c2", bufs=3) as c2p,
                tc.tile_pool(name="c2ps", bufs=3, space="PSUM") as c2ps,
            ):
                for r in range(32):
                    tsl = slice(r * 128, (r + 1) * 128)
                    o2 = c2p.tile([128, 256], b16, tag="o2")
                    nc.sync.dma_start(o2[:], ot[tsl, :])
                    for h in range(2):
                        hsl2 = slice(h * 128, (h + 1) * 128)
                        osq = c2p.tile([128, 128], b16, tag="osq")
                        ss = c2p.tile([128, 1], f32, tag="ss")
                        nc.scalar.activation(
                            osq[:], o2[:, hsl2], AF.Square, accum_out=ss[:]
                        )
                        rt2 = c2p.tile([128, 1], f32, tag="rt2")
                        nc.scalar.activation(
                            rt2[:], ss[:], AF.Sqrt, bias=epst[:, 0:1], scale=1.0 / D
                        )
                        rr2 = c2p.tile([128, 1], f32, tag="rr2")
                        nc.vector.reciprocal(rr2[:], rt2[:])
                        o_n = c2p.tile([128, 128], b16, tag="o_n")
                        nc.vector.tensor_scalar_mul(o_n[:], o2[:, hsl2], rr2[:])
                        onp = c2ps.tile([128, 128], b16, tag="onp")
                        nc.tensor.transpose(onp[:], o_n[:], idb_sb[:])
                        sgt = c2p.tile([128, 128], b16, tag="sgt")
                        nc.sync.dma_start(sgt[:], sg[hsl2, tsl])
                        ogt2 = c2p.tile([128, 128], b16, tag="ogt2")
                        nc.vector.tensor_mul(ogt2[:], onp[:], sgt[:])
                        nc.sync.dma_start(og[hsl2, tsl], ogt2[:])

            # ================= Phase D: output projection =================
            with (
                tc.tile_pool(name="op", bufs=4) as opool,
                tc.tile_pool(name="opps", bufs=4, space="PSUM") as oppool,
                tc.tile_pool(name="wop", bufs=1) as wopool,
            ):
                wo_t = []
                for c2 in range(2):
                    wot = wopool.tile([128, HID], b16, tag=f"wo{c2}")
                    nc.sync.dma_start(wot[:], wo[c2 * 128:(c2 + 1) * 128, :])
                    wo_t.append(wot)
                for tt in range(32):
                    ogt = []
                    for c2 in range(2):
                        t = opool.tile([128, 128], b16, tag="ogtile")
                        nc.sync.dma_start(
                            t[:], og[c2 * 128:(c2 + 1) * 128, tt * 128:(tt + 1) * 128]
                        )
                        ogt.append(t)
                    outsb = opool.tile([128, HID], b16, tag="outsb")
                    for n4 in range(4):
                        ps = oppool.tile([128, 512], f32, tag="ops")
                        for c2 in range(2):
                            nc.tensor.matmul(
                                ps[:], ogt[c2][:],
                                wo_t[c2][:, n4 * 512:(n4 + 1) * 512],
                                start=(c2 == 0), stop=(c2 == 1),
                            )
                        nc.scalar.copy(outsb[:, n4 * 512:(n4 + 1) * 512], ps[:])
                    nc.sync.dma_start(opf[tt * 128:(tt + 1) * 128, :], outsb[:])
                nc.gpsimd.collective_compute(
                    "ReduceScatter", mybir.AluOpType.add,
                    replica_groups=[list(range(8))],
                    ins=[opf[:]], outs=[rso[:]],
                )
                nc.sync.dma_start(out[:], rso[:])

    nc.compile()
    return nc


def _chmaj(w):  # [256, 4] -> [128, 8] (ch-chunk along cols)
    return np.ascontiguousarray(w.reshape(2, 128, 4).transpose(1, 0, 2).reshape(128, 8)).astype(np.float32)


def _prep_inputs(inputs):
    f32 = np.float32
    hs = np.asarray(inputs["hidden_states"], f32).reshape(N, HID)
    hst = np.ascontiguousarray(hs.T).astype(bf16)       # [HID, N]

    def tile_w(w_t):  # [HID, M] -> [128, 16*M] kt-tiled
        m = w_t.shape[1]
        return np.ascontiguousarray(
            w_t.reshape(16, 128, m).transpose(1, 0, 2).reshape(128, 16 * m)
        ).astype(bf16)

    Wq, Wk, Wv = (np.asarray(inputs[x], f32) for x in ("Wq", "Wk", "Wv"))
    Wfa, Wfb = np.asarray(inputs["Wfa"], f32), np.asarray(inputs["Wfb"], f32)
    Wga, Wgb = np.asarray(inputs["Wga"], f32), np.asarray(inputs["Wgb"], f32)
    Wo = np.asarray(inputs["Wo"], f32)
    onw = np.asarray(inputs["o_norm_w"], f32)
    Wo_fold = Wo * np.tile(onw, H)[None, :]
    A = np.asarray(inputs["A_log"], f32).reshape(H)
    dt_bias = np.asarray(inputs["dt_bias"], f32)
    beta_all = 1.0 / (1.0 + np.exp(-(hs @ np.asarray(inputs["Wb"], f32).T)))  # [N, H]
    gpre = (hs @ Wfa.T) @ Wfb.T + dt_bias[None, :]
    aneg_full = np.repeat(-np.exp(A), D)[None, :]
    g_full = (aneg_full * np.logaddexp(0.0, gpre)).astype(f32)     # [N, P]

    Lx = np.triu(np.ones((128, 128), f32))              # L[j,t] = 1 if j <= t
    mS = np.tril(np.ones((128, 128), f32), -1).astype(bf16)
    mI = np.tril(np.ones((128, 128), f32), 0).astype(bf16)
    idb = np.eye(128, dtype=f32).astype(bf16)
    idf = np.eye(128, dtype=f32)
    pm1 = np.array([[1.0, -1.0]], f32)

    in_maps = []
    for core in range(8):
        cs = slice(core * 256, (core + 1) * 256)
        hsl = slice(core * 2, core * 2 + 2)
        im = {
            "hsts": np.ascontiguousarray(hst[:, core * 512:(core + 1) * 512]),
            "wq": tile_w(np.ascontiguousarray(Wq[cs].T)),
            "wk": tile_w(np.ascontiguousarray(Wk[cs].T)),
            "wv": tile_w(np.ascontiguousarray(Wv[cs].T)),
            "wga": tile_w(np.ascontiguousarray(Wga.T)),
            "wgb": np.ascontiguousarray(Wgb[cs].T).astype(bf16),
            "gs": np.ascontiguousarray(g_full[:, cs].T),
            "wo": np.ascontiguousarray(Wo_fold[:, cs].T).astype(bf16),
            "cwq": _chmaj(np.asarray(inputs["wq_conv"], f32)[cs]),
            "cwk": _chmaj(np.asarray(inputs["wk_conv"], f32)[cs]),
            "cwv": _chmaj(np.asarray(inputs["wv_conv"], f32)[cs]),
            "betah": np.ascontiguousarray(beta_all[:, hsl].T).astype(f32),
            "lx": Lx, "maskS": mS, "maskI": mI, "idb": idb, "idf": idf, "pm1": pm1,
        }
        in_maps.append(im)
    return in_maps


def _get_nc():
    if "nc" not in _CACHED:
        _CACHED["nc"] = _build_nc()
    return _CACHED["nc"]


def kernel(**inputs):
    _config_jax_cache()
    from concourse.bass_utils import run_bass_kernel_spmd

    nc = _get_nc()
    in_maps = _prep_inputs(inputs)
    res = run_bass_kernel_spmd(nc, in_maps, list(range(8)))
    acc = np.concatenate(
        [np.asarray(r["out"], dtype=np.float32) for r in res.results], axis=0
    )
    return acc.reshape(B, T, HID)
